# revision 99
# baseline (speedup 1.0000x reference)
"""Trainium2 Bass kernel for a dense transformer block (PreNorm attn + MLP).

Full inputs: x [8, 1024, 768] f32 + LN/attn/MLP weights.
Sharding: pure data-parallel — batch 8 across 8 NeuronCores, no collectives.

Per-core design (tokens n=1024, d=768, heads=12, dh=64, hidden=3072):
  - Residual spine fp32 (x2) / bf16 (x), FEATURE-major; weights [d_in, d_out]
    serve as lhsT directly.
  - Deep matmuls fp8e4 DoubleRow (0.5 cyc/row contracting two 128-k-tiles);
    fc1 is MIXED: hidden 0:1536 fp8 DR (weights x16, gelu scale 1/16),
    1536:3072 bf16 — full-fp8 fc1 would blow the 2e-2 error budget.
  - Scores: q/k "folded" — head h on 32 partitions, dh split into the 2
    DoubleRow k-tiles (host-side qkv_w column permutation makes this free).
  - attnV: lhsT = v_aug [j, 2, 96]: 64 v dims + ones col (softmax
    denominator) + 31 zero pad; v bias folded into proj_b host-side.
    1/denom broadcast via PE ones-matmul + DVE copy (gpsimd can't touch
    PSUM; 0-stride-partition SBUF DMA is rejected).
  - LN1 is TOKEN-major: bn_stats/bn_aggr per 128-token block as x lands,
    h1 = tensor_scalar((x-mu)*rstd) per block, then PE-transposed to
    feature-major (x first — it only needs the DMA; h1 second).  Kills the
    old stats->broadcast->apply chain.  pbsum is folded into muxr.
  - exp on ACT is THE bottleneck (~100us); scheduling keeps ACT hot:
    ic0 MLP (proj/LN2/fc1) drains between score-pairs of ic1's exp stream
    (bf16 fc1 split 256-wide to fit the 1.04us exp cadence); the 12 fp8
    fc1(ic0) chunks + 4 bf16 quarters are emitted in the attnV flush so
    their gelus seamlessly extend the exp stream.
  - Tail: attention PSUM pools close, a 6-buf ps_tail pool opens (psum
    rotation was the pacing limit); LN2 mu from the wsum trick before proj,
    s2/var fused via ACT Square + stt, f32 PE broadcast; band0 (256 tok)
    stats fast-path + full-width pass for band1; band0 gelus into a buffer
    aliased on dead v_aug so fc1(ic1) needn't wait for fc2(ic0); band1 fc1
    interleaves with fc2(ic1) tq0 to keep the gelu stream continuous.
  - DMAs coalesced (26 total): x in 4, each weight matrix 1-2, biases
    packed into one [128,49] tensor host-side.
"""

import collections

import numpy as np
import ml_dtypes

import concourse.bass as bass
import concourse.tile as tile
from concourse import mybir
from concourse.masks import make_identity
from concourse.bass_utils import run_bass_kernel_spmd

F32 = mybir.dt.float32
BF16 = mybir.dt.bfloat16
FP8 = mybir.dt.float8e4
BF16_NP = ml_dtypes.bfloat16
FP8_NP = ml_dtypes.float8_e4m3
AF = mybir.ActivationFunctionType
DR = mybir.MatmulPerfMode.DoubleRow
ADD = mybir.AluOpType.add

N = 1024          # tokens per core
D = 768           # model dim
H = 12            # heads
DH = 64           # head dim
HID = 3072        # mlp hidden
P = 128
NT = N // P       # 8 token chunks
DC = D // P       # 6 feature chunks
HC = HID // P     # 24 hidden chunks
KP = DC // 2      # 3 contraction pairs for d=768
IC = 2            # token halves of 512
VA = 96           # attnV stationary cols: 64 v + ones + 31 pad
EPS = 1e-5


def build_nc(qkb_zero=False):
    nc = bass.Bass("TRN2")

    x_d = nc.dram_tensor("x", [N, D], BF16, kind="ExternalInput")
    wqkv_d = nc.dram_tensor("wqkv", [D, 3 * D], FP8, kind="ExternalInput")
    projw_d = nc.dram_tensor("proj_w", [D, D], FP8, kind="ExternalInput")
    wsum_d = nc.dram_tensor("wsum8", [D, 32], FP8, kind="ExternalInput")
    biases_d = nc.dram_tensor("biases", [49 * P], F32, kind="ExternalInput")
    fc1w_d = nc.dram_tensor("fc1_w", [D, HID // 2], BF16,
                            kind="ExternalInput")
    fc1w8_d = nc.dram_tensor("fc1_w8", [D, HID // 2], FP8,
                             kind="ExternalInput")
    fc2w_d = nc.dram_tensor("fc2_w", [HID, D], FP8, kind="ExternalInput")
    y_d = nc.dram_tensor("y", [N, D], F32, kind="ExternalOutput")

    with tile.TileContext(nc) as tc:
        _body(nc, tc, x_d, wqkv_d, biases_d, projw_d,
              fc1w_d, fc1w8_d, fc2w_d, y_d, qkb_zero, wsum_d)
    # this container's walrus accepts at most 1 sync wait per instruction
    # (2 on EventSemaphore); redistribute excess waits like Bacc.compile does
    import bass_rust as _br
    _br.move_matmul_waits_to_ldweights(nc.m)
    _br.generate_event_semaphores(nc)
    return nc


def _body(nc, tc, x_d, wqkv_d, biases_d, projw_d,
          fc1w_d, fc1w8_d, fc2w_d, y_d, qkb_zero, wsum_d):
    from contextlib import ExitStack
    with ExitStack() as ctx:
        consts = ctx.enter_context(tc.tile_pool(name="consts", bufs=1))
        rows = ctx.enter_context(tc.tile_pool(name="rows", bufs=3))
        recp = ctx.enter_context(tc.tile_pool(name="recp", bufs=2))
        rbp = ctx.enter_context(tc.tile_pool(name="rbp", bufs=2))
        ltp = ctx.enter_context(tc.tile_pool(name="ltp", bufs=2))
        ftp = ctx.enter_context(tc.tile_pool(name="ftp", bufs=1))
        bcp = ctx.enter_context(tc.tile_pool(name="bcp", bufs=2))
        dram = ctx.enter_context(tc.tile_pool(name="dram", bufs=1, space="DRAM"))
        ps_mm = ctx.enter_context(tc.tile_pool(name="ps_mm", bufs=2, space="PSUM"))
        ps_ref = [ps_mm]
        mem = ctx.enter_context(tc.tile_pool(name="mem", bufs=1))
        mem2 = ctx.enter_context(tc.tile_pool(name="mem2", bufs=2))

        ALP = nc.allow_low_precision

        # ---- constants & early DMAs (x first, then q/k weights) ----
        ident = consts.tile([P, P], F32, tag="ident")
        make_identity(nc, ident)
        ident_bf = consts.tile([P, P], BF16, tag="ident_bf")
        make_identity(nc, ident_bf)

        x_tok = mem.tile([P, NT, D], BF16, tag="xo")           # -> out_fm later
        x_view = x_d[:, :].rearrange("(t p) d -> p t d", p=P)
        for tp in range(4):
            nc.sync.dma_start(out=x_tok[:, 2 * tp:2 * tp + 2, :],
                              in_=x_view[:, 2 * tp:2 * tp + 2, :])

        wqkv_sb = mem.tile([P, DC, 3 * D], FP8, tag="w1")      # -> fc1w later
        wqkv_v = wqkv_d[:, :].rearrange("(ko p) m -> p ko m", p=P)
        nc.sync.dma_start(out=wqkv_sb[:, :, 0:2 * D],
                          in_=wqkv_v[:, :, 0:2 * D])

        ones8 = consts.tile([P, 2, 32], FP8, tag="ones8")
        nc.vector.memset(ones8, 1.0)
        onesb_c = consts.tile([P, 1], BF16, tag="onesb_c")     # stats lhsT
        nc.vector.memset(onesb_c, 1.0)
        onesb_r = consts.tile([1, P], BF16, tag="onesb_r")     # bcast lhsT
        nc.vector.memset(onesb_r, 1.0)
        onesb_rf = consts.tile([1, P], F32, tag="onesb_rf")    # f32 bcast
        nc.vector.memset(onesb_rf, 1.0)
        eps_sb = consts.tile([P, 1], F32, tag="eps_sb")
        nc.vector.memset(eps_sb, EPS)

        # biases packed host-side: cols 0:12 qkb, 12:18 projb, 18:42 fc1b,
        # 42:48 fc2b, 48 pbsum (broadcast)
        bias_all = consts.tile([P, 49], F32, tag="biases")
        nc.sync.dma_start(out=bias_all,
                          in_=biases_d[:].rearrange("(mo p) -> p mo", p=P))
        qkb_sb = bias_all[:, 0:12]
        projb_sb = bias_all[:, 12:18]
        fc1b_sb = bias_all[:, 18:42]
        fc2b_sb = bias_all[:, 42:48]
        pbs_sb = bias_all[0:1, 48:49]

        # deferred weight DMAs (after x / qk in the DMA queues)
        nc.sync.dma_start(out=wqkv_sb[:, :, 2 * D:],
                          in_=wqkv_v[:, :, 2 * D:])
        wsum_sb = consts.tile([P, DC, 32], FP8, tag="wsum")
        nc.sync.dma_start(out=wsum_sb,
                          in_=wsum_d[:, :].rearrange("(ko p) m -> p ko m", p=P))
        projw_sb = mem.tile([P, DC, D], FP8, tag="pw")
        projw_v = projw_d[:, :].rearrange("(ko p) m -> p ko m", p=P)
        nc.sync.dma_start(out=projw_sb, in_=projw_v)

        # v_aug: ones col + zero pad (finite garbage would still poison psum)
        v_aug = mem.tile([P, NT, H, VA], FP8, tag="vf")

        # ---- lead-in: token-major LN1 (per-token stats on free axis) ----
        x_fm = mem.tile([P, DC, N], BF16, tag="xf")
        muxr = mem.tile([1, N], BF16, tag="mux")
        h1 = mem.tile([P, DC, N], FP8, tag="ha")               # -> h2 later
        h1_tok = mem.tile([P, NT, D], BF16, tag="ge")      # -> gelu_t later
        mv_tok = mem.tile([P, NT, 2], F32, tag="mvt")
        rstd_tok = mem.tile([P, NT], F32, tag="rst")

        SUB = mybir.AluOpType.subtract
        MUL = mybir.AluOpType.mult
        for t in range(NT):
            bs = rows.tile([P, 2, 6], F32, tag="bs", name="bn")
            nc.vector.bn_stats(bs[:, 0, :], x_tok[:, t, 0:384])
            nc.vector.bn_stats(bs[:, 1, :], x_tok[:, t, 384:768])
            nc.vector.bn_aggr(mv_tok[:, t, :], bs)
            if t % 4 == 3:
                hf = t // 4
                nc.scalar.activation(
                    out=rstd_tok[:, 4 * hf:4 * hf + 4],
                    in_=mv_tok[:, 4 * hf:4 * hf + 4, 1],
                    func=AF.Sqrt, bias=eps_sb, scale=1.0)
                nc.vector.reciprocal(
                    out=rstd_tok[:, 4 * hf:4 * hf + 4],
                    in_=rstd_tok[:, 4 * hf:4 * hf + 4])
                for tt in range(4 * hf, 4 * hf + 4):
                    eng = nc.vector if tt % 2 else nc.gpsimd
                    with ALP(reason="fp8 h1"):
                        eng.tensor_scalar(
                            out=h1_tok[:, tt, :], in0=x_tok[:, tt, :],
                            scalar1=mv_tok[:, tt, 0:1],
                            scalar2=rstd_tok[:, tt:tt + 1],
                            op0=SUB, op1=MUL)

        nc.gpsimd.memset(v_aug[:, :, :, DH + 1:], 0.0)
        nc.gpsimd.memset(v_aug[:, :, :, DH:DH + 1], 1.0)
        # transposes to feature-major: x first (only needs the DMA), h1
        # second (gated by the per-token stats chain)
        with tc.tile_pool(name="ps_lead", bufs=4, space="PSUM") as ps_lead:
            for hf in range(2):
                sl = slice(hf * 512, (hf + 1) * 512)
                for dc in range(DC):
                    pt = ps_lead.tile([P, 4, P], BF16, tag="tr", name="ptx")
                    for q in range(4):
                        t = hf * 4 + q
                        nc.tensor.transpose(
                            pt[:, q, :], x_tok[:, t, dc * P:(dc + 1) * P],
                            ident_bf)
                    with ALP(reason="bf16 x_fm"):
                        nc.scalar.copy(
                            out=x_fm[:, dc, sl],
                            in_=pt.rearrange("p a b -> p (a b)"))
            for hf in range(2):
                sl = slice(hf * 512, (hf + 1) * 512)
                for dc in range(DC):
                    ph = ps_lead.tile([P, 4, P], BF16, tag="tr", name="pth")
                    for q in range(4):
                        t = hf * 4 + q
                        nc.tensor.transpose(
                            ph[:, q, :], h1_tok[:, t, dc * P:(dc + 1) * P],
                            ident_bf)
                    heng = nc.scalar.copy if dc % 2 else nc.vector.tensor_copy
                    with ALP(reason="fp8 h1 fm"):
                        heng(out=h1[:, dc, sl],
                             in_=ph.rearrange("p a b -> p (a b)"))
                # muxr row = mean_d(x) + pbsum (only needed by LN2, late)
                psm = ps_lead.tile([1, 512], F32, tag="tr", name="ps_mu1")
                for k in range(DC):
                    nc.tensor.matmul(psm, onesb_c, x_fm[:, k, sl],
                                     start=(k == 0), stop=(k == DC - 1))
                with ALP(reason="bf16 mux"):
                    nc.vector.tensor_scalar(out=muxr[0:1, sl], in0=psm,
                                            scalar1=1.0 / D,
                                            scalar2=pbs_sb,
                                            op0=mybir.AluOpType.mult, op1=ADD)

        # ---- DoubleRow helpers ----
        def dr_group(ps_ap, lhs_fn, rhs_fn, nkp):
            for kp in range(nkp):
                nc.tensor.matmul(ps_ap, lhs_fn(kp), rhs_fn(kp),
                                 start=(kp == 0), stop=(kp == nkp - 1),
                                 perf_mode=DR)

        q_fold = mem.tile([P, 3, 2, N], FP8, tag="qf")
        k_fold = mem.tile([P, 3, 2, N], FP8, tag="kf")

        def emit_qk_chunk(j, ic, act=False):
            """j in 0..11: q chunks 0-5 as (g, half), k chunks 6-11."""
            g, half = divmod(j % 6, 2)
            dst = k_fold if j >= 6 else q_fold
            sl = slice(ic * 512, (ic + 1) * 512)
            ps = ps_ref[0].tile([P, 512], F32, tag="mm", name="ps_qk")
            dr_group(ps,
                     lambda kp: wqkv_sb[:, 2 * kp:2 * kp + 2, j * P:(j + 1) * P],
                     lambda kp: h1[:, 2 * kp:2 * kp + 2, sl], KP)
            with ALP(reason="fp8 qk"):
                if act and qkb_zero:
                    nc.scalar.copy(out=dst[:, g, half, sl], in_=ps)
                elif qkb_zero:
                    nc.vector.tensor_copy(out=dst[:, g, half, sl], in_=ps)
                else:
                    nc.vector.tensor_scalar_add(out=dst[:, g, half, sl],
                                                in0=ps,
                                                scalar1=qkb_sb[:, j:j + 1])

        def emit_v_chunk(t, vc):
            fw = 512 if vc == 0 else 256
            ps = ps_ref[0].tile([P, 512], F32, tag="mm", name="ps_v")
            dr_group(ps[:, :fw],
                     lambda kp: h1[:, 2 * kp:2 * kp + 2, t * P:(t + 1) * P],
                     lambda kp: wqkv_sb[:, 2 * kp:2 * kp + 2,
                                        2 * D + vc * 512:2 * D + vc * 512 + fw],
                     KP)
            with ALP(reason="fp8 v"):
                nc.vector.tensor_copy(
                    out=v_aug[:, t, vc * 8:vc * 8 + fw // DH, 0:DH],
                    in_=ps[:, :fw].rearrange("p (h e) -> p h e", e=DH))

        fc1w_sb = mem.tile([P, DC, HID // 2], BF16, tag="w1")
        fc1w_v = fc1w_d[:, :].rearrange("(ko p) m -> p ko m", p=P)
        fc1w8_sb = mem.tile([P, DC, HID // 2], FP8, tag="w18")
        fc1w8_v = fc1w8_d[:, :].rearrange("(ko p) m -> p ko m", p=P)
        fc2w_sb = mem.tile([P, HC, D], FP8, tag="f2")
        fc2w_v = fc2w_d[:, :].rearrange("(ko p) m -> p ko m", p=P)

        x2_fm = mem.tile([P, DC, N], F32, tag="x2")
        attn_fm = mem.tile([P, DC, N], FP8, tag="at")
        x2s = mem.tile([P, DC, 512], FP8, tag="xq")
        gelu_t = mem.tile([P, HC, 512], FP8, tag="ge")
        out_fm = mem.tile([P, DC, N], F32, tag="xo")

        def emit_proj_chunk(ic, mo, q0, qw):
            """token window [ic*512+q0, +qw); x2c/x2s live at [q0, q0+qw)."""
            sl = slice(ic * 512 + q0, ic * 512 + q0 + qw)
            sq = slice(q0, q0 + qw)
            ps = ps_ref[0].tile([P, 512], F32, tag="mm", name="ps_proj")
            dr_group(ps[:, :qw],
                     lambda kp: projw_sb[:, 2 * kp:2 * kp + 2,
                                         mo * P:(mo + 1) * P],
                     lambda kp: attn_fm[:, 2 * kp:2 * kp + 2, sl], KP)
            nc.vector.scalar_tensor_tensor(
                out=x2_fm[:, mo, sl], in0=ps[:, :qw],
                scalar=projb_sb[:, mo:mo + 1], in1=x_fm[:, mo, sl],
                op0=ADD, op1=ADD)
            with ALP(reason="fp8 stats"):
                enq = nc.gpsimd if mo % 2 else nc.vector
                enq.tensor_mul(x2s[:, mo, sq], x2_fm[:, mo, sl],
                               x2_fm[:, mo, sl])

        def emit_ln2_mu(ic, q0, qw):
            gsl = slice(ic * 512 + q0, ic * 512 + q0 + qw)
            murow = rows.tile([1, 512], F32, tag="row", name="mu2row")
            psr = ps_ref[0].tile([32, 512], F32, tag="mm", name="ps_mu2")
            dr_group(psr[:, :qw], lambda kp: wsum_sb[:, 2 * kp:2 * kp + 2, :],
                     lambda kp: attn_fm[:, 2 * kp:2 * kp + 2, gsl], KP)
            # muxr carries mean_d(x) + pbsum (folded at lead-in)
            nc.vector.scalar_tensor_tensor(
                out=murow[:, :qw], in0=psr[0:1, :qw], scalar=1.0 / D,
                in1=muxr[0:1, gsl], op0=mybir.AluOpType.mult, op1=ADD)
            mu_bf = rows.tile([1, 512], BF16, tag="rowb", name="mu2bf")
            with ALP(reason="bf16 rows"):
                nc.vector.tensor_copy(out=mu_bf[:, :qw], in_=murow[:, :qw])
            mu2_bc = bcp.tile([P, 512], F32, tag="bc", name="mu2_bc")
            psb1 = ps_ref[0].tile([P, 512], F32, tag="mm", name="psb_mu2")
            nc.tensor.matmul(psb1[:, :qw], onesb_r, mu_bf[:, :qw],
                             start=True, stop=True)
            nc.vector.tensor_copy(out=mu2_bc[:, :qw], in_=psb1[:, :qw])
            return murow, mu2_bc

        def emit_ln2_s2(ic, q0, qw, murow, mu2_bc):
            sq = slice(q0, q0 + qw)
            # mu^2 on ACT (idle in the tail window); var fused via stt
            musq = rows.tile([1, 512], F32, tag="row", name="musq")
            nc.scalar.activation(out=musq[:, :qw], in_=murow[:, :qw],
                                 func=AF.Square, scale=1.0)
            pss = ps_ref[0].tile([32, 512], F32, tag="mm", name="ps_s22")
            dr_group(pss[:, :qw], lambda kp: ones8,
                     lambda kp: x2s[:, 2 * kp:2 * kp + 2, sq], KP)
            var = rows.tile([1, 512], F32, tag="row", name="var2")
            nc.vector.scalar_tensor_tensor(
                out=var[:, :qw], in0=pss[0:1, :qw], scalar=1.0 / D,
                in1=musq[:, :qw], op0=mybir.AluOpType.mult,
                op1=mybir.AluOpType.subtract)
            rstd2 = rows.tile([1, 512], F32, tag="row", name="rstd2")
            nc.scalar.activation(out=rstd2[:, :qw], in_=var[:, :qw],
                                 func=AF.Sqrt, bias=eps_sb[0:1, :], scale=1.0)
            nc.vector.reciprocal(out=rstd2[:, :qw], in_=rstd2[:, :qw])
            rstd2_bc = bcp.tile([P, 512], F32, tag="bc", name="rstd2_bc")
            psb2 = ps_ref[0].tile([P, 512], F32, tag="mm", name="psb_rs2")
            nc.tensor.matmul(psb2[:, :qw], onesb_rf, rstd2[:, :qw],
                             start=True, stop=True)
            nc.vector.tensor_copy(out=rstd2_bc[:, :qw], in_=psb2[:, :qw])
            return mu2_bc, rstd2_bc

        def emit_ln2_stats(ic, q0, qw):
            murow, mu2_bc = emit_ln2_mu(ic, q0, qw)
            return emit_ln2_s2(ic, q0, qw, murow, mu2_bc)

        h2 = [None, None]
        h2_8 = [None, None]

        def emit_ln2_apply(ic, bcs, q0, qw, dcs):
            mu2_bc, rstd2_bc = bcs
            sl = slice(ic * 512 + q0, ic * 512 + q0 + qw)
            sq = slice(q0, q0 + qw)
            if h2[ic] is None:
                h2[ic] = mem.tile([P, DC, 512], BF16, tag="ha", name=f"h2_{ic}")
                h2_8[ic] = mem.tile([P, DC, 512], FP8, tag="h28",
                                    name=f"h28_{ic}")
            for dc in dcs:
                engA, engB = ((nc.vector, nc.gpsimd) if dc % 2 == 0
                              else (nc.gpsimd, nc.vector))
                lt = ltp.tile([P, 512], BF16, tag="lt", name="ln2_tmp")
                with ALP(reason="ln2"):
                    engA.tensor_sub(lt[:, :qw], x2_fm[:, dc, sl],
                                    mu2_bc[:, q0:q0 + qw])
                    engA.tensor_mul(h2[ic][:, dc, sq], lt[:, :qw],
                                    rstd2_bc[:, q0:q0 + qw])
                    engB.tensor_mul(h2_8[ic][:, dc, sq], lt[:, :qw],
                                    rstd2_bc[:, q0:q0 + qw])

        def emit_fc1_chunk(ic, mo, q0=0, qw=512, gdst=None):
            sq = slice(q0, q0 + qw)
            ps = ps_ref[0].tile([P, 512], F32, tag="mm", name="ps_fc1")
            if mo < HC // 2:
                dr_group(ps[:, :qw],
                         lambda kp: fc1w8_sb[:, 2 * kp:2 * kp + 2,
                                             mo * P:(mo + 1) * P],
                         lambda kp: h2_8[ic][:, 2 * kp:2 * kp + 2, sq], KP)
                scale = 1.0 / 16.0
            else:
                mb = mo - HC // 2
                for k in range(DC):
                    nc.tensor.matmul(ps[:, :qw],
                                     fc1w_sb[:, k, mb * P:(mb + 1) * P],
                                     h2[ic][:, k, sq],
                                     start=(k == 0), stop=(k == DC - 1))
                scale = 1.0
            if gdst is None:
                gdst = gelu_t[:, mo, sq]
            with ALP(reason="fp8 gelu"):
                nc.scalar.activation(out=gdst, in_=ps[:, :qw],
                                     func=AF.Gelu,
                                     bias=fc1b_sb[:, mo:mo + 1], scale=scale)

        def emit_fc2_chunk(ic, mo, q0, qw, gsrc=None):
            sl = slice(ic * 512 + q0, ic * 512 + q0 + qw)
            sq = slice(q0, q0 + qw)
            if gsrc is None:
                gsrc = lambda kp: gelu_t[:, 2 * kp:2 * kp + 2, sq]
            ps = ps_ref[0].tile([P, 512], F32, tag="mm", name="ps_fc2")
            dr_group(ps[:, :qw],
                     lambda kp: fc2w_sb[:, 2 * kp:2 * kp + 2,
                                        mo * P:(mo + 1) * P],
                     gsrc, HC // 2)
            ft = ftp.tile([P, 512], BF16, tag="ft", name="fc2_tmp")
            with ALP(reason="bf16 fc2 tmp"):
                nc.vector.tensor_scalar(out=ft[:, :qw], in0=ps[:, :qw],
                                        scalar1=1.0 / 16.0,
                                        scalar2=fc2b_sb[:, mo:mo + 1],
                                        op0=mybir.AluOpType.mult, op1=ADD)
            nc.gpsimd.tensor_add(out_fm[:, mo, sl], ft[:, :qw],
                                 x2_fm[:, mo, sl])

        def emit_exit_tr(t, tail=False):
            y_stage = mem2.tile([P, D], F32, tag="ys", name="y_stage")
            for dg in range(2):
                pt = ps_ref[0].tile([P, 3, P], F32, tag="mm", name="ps_tr2")
                for q in range(3):
                    dc = dg * 3 + q
                    nc.tensor.transpose(pt[:, q, :],
                                        out_fm[:, dc, t * P:(t + 1) * P],
                                        ident)
                eng = nc.scalar.copy if tail and dg % 2 \
                    else nc.vector.tensor_copy
                eng(out=y_stage[:, dg * 3 * P:(dg + 1) * 3 * P],
                    in_=pt.rearrange("p a b -> p (a b)"))
            nc.sync.dma_start(out=y_d[t * P:(t + 1) * P, :], in_=y_stage)

        # ---- attention + work-queue schedule ----
        wq = collections.deque()

        def drain(n):
            for _ in range(min(n, len(wq))):
                wq.popleft()()

        def refill(ic, h):
            if ic == 0:
                if h == 0:
                    for t in range(NT):
                        for vc in range(2):
                            wq.append(lambda t=t, vc=vc: emit_v_chunk(t, vc))
                elif h == 1:
                    for j in (2, 3, 8, 9):
                        for i2 in range(IC):
                            wq.append(lambda j=j, i2=i2: emit_qk_chunk(j, i2))
                elif h == 2:
                    for j in (4, 5, 10, 11):
                        for i2 in range(IC):
                            wq.append(lambda j=j, i2=i2: emit_qk_chunk(j, i2))
                if h == 7:
                    nc.sync.dma_start(out=fc1w8_sb, in_=fc1w8_v)
                elif h in (8, 9):
                    ko = 3 * (h - 8)
                    nc.sync.dma_start(out=fc1w_sb[:, ko:ko + 3, :],
                                      in_=fc1w_v[:, ko:ko + 3, :])
                elif h == 11:
                    nc.sync.dma_start(out=fc2w_sb[:, 0:4, :],
                                      in_=fc2w_v[:, 0:4, :])
            else:
                if h < 5:
                    ko = 4 * h + 4
                    nc.sync.dma_start(out=fc2w_sb[:, ko:ko + 4, :],
                                      in_=fc2w_v[:, ko:ko + 4, :])
                if h == 1:
                    # attn_fm(ic0) complete once attnV(h11, ic0) drained (h0)
                    for mo in range(DC):
                        wq.append(lambda mo=mo: emit_proj_chunk(0, mo, 0, 512))
                elif h == 2:
                    def stats0():
                        _st["bcs0"] = emit_ln2_stats(0, 0, 512)
                    wq.append(stats0)
                    wq.append(lambda: emit_ln2_apply(0, _st["bcs0"], 0, 512,
                                                     range(3)))
                    wq.append(lambda: emit_ln2_apply(0, _st["bcs0"], 0, 512,
                                                     range(3, DC)))
                elif h in (3, 4, 5, 6, 7, 8):
                    hi = 12 + 2 * (h - 3)
                    for mo in range(hi, min(hi + 2, 22)):
                        for q in range(2):
                            wq.append(lambda mo=mo, q=q:
                                      emit_fc1_chunk(0, mo, 256 * q, 256))

        def emit_attnv(h, ic, expT):
            pso = ps_att.tile([VA, 512], F32, tag="att", name="pso")
            for c in range(NT // 2):
                nc.tensor.matmul(pso, v_aug[:, 2 * c:2 * c + 2, h, :],
                                 expT[:, 2 * c:2 * c + 2, :],
                                 start=(c == 0), stop=(c == NT // 2 - 1),
                                 perf_mode=DR)
            rec = recp.tile([1, 512], BF16, tag="rec", name="rec")
            with ALP(reason="bf16 recip"):
                nc.vector.reciprocal(out=rec, in_=pso[DH:DH + 1, :])
            # PE broadcast of 1/denom into PSUM, DVE copy to SBUF
            psb = ps_ref[0].tile([P, 512], F32, tag="mm", name="psb")
            nc.tensor.matmul(psb[0:DH, :], onesb_r[:, 0:DH], rec,
                             start=True, stop=True)
            rb = rbp.tile([DH, 512], BF16, tag="rb", name="rb")
            with ALP(reason="bf16 rb"):
                nc.vector.tensor_copy(out=rb, in_=psb[0:DH, :])
            with ALP(reason="fp8 attn"):
                nc.vector.tensor_mul(
                    out=attn_fm[64 * (h % 2):64 * (h % 2) + 64,
                                h // 2, ic * 512:(ic + 1) * 512],
                    in0=pso[0:DH, :], in1=rb)

        _st = {}
        DEPTH = 1  # attnV(h) emitted after scores(h+DEPTH)
        with tc.tile_pool(name="ps_sc", bufs=2, space="PSUM") as ps_sc, \
             tc.tile_pool(name="ps_att", bufs=2, space="PSUM") as ps_att, \
             tc.tile_pool(name="expp", bufs=DEPTH + 1) as expp:
            for j in (6, 7, 0, 1):
                for i2 in range(IC):
                    emit_qk_chunk(j, i2, act=(j >= 6))
            pend = collections.deque()
            for ic in range(IC):
                for h in range(H):
                    refill(ic, h)
                    g, b = divmod(h, 4)
                    p0 = 32 * b
                    expT = expp.tile([P, NT, 512], FP8, tag="ex", name="expT")
                    for jp in range(NT // 2):
                        ps = ps_sc.tile([P, 2, 512], F32, tag="sc",
                                        name="ps_sc")
                        for half in range(2):
                            jc = 2 * jp + half
                            nc.tensor.matmul(
                                ps[:, half, :],
                                k_fold[p0:p0 + 32, g, :, jc * P:(jc + 1) * P],
                                q_fold[p0:p0 + 32, g, :,
                                       ic * 512:(ic + 1) * 512],
                                start=True, stop=True, perf_mode=DR,
                                tile_position=(p0, 0))
                        with ALP(reason="fp8 exp"):
                            nc.scalar.activation(
                                out=expT[:, 2 * jp:2 * jp + 2, :], in_=ps,
                                func=AF.Exp, scale=0.125)
                        drain(2 if h < 5 else 1)
                    pend.append((h, ic, expT))
                    if len(pend) > DEPTH:
                        emit_attnv(*pend.popleft())
                    drain(2)
            dq = collections.deque(
                list(range(12)) + [(22, 0), (22, 1), (23, 0), (23, 1)])
            while pend:
                emit_attnv(*pend.popleft())
                drain(2)
                for _ in range(4):
                    if dq:
                        it = dq.popleft()
                        if isinstance(it, tuple):
                            emit_fc1_chunk(0, it[0], 256 * it[1], 256)
                        else:
                            emit_fc1_chunk(0, it)
            drain(len(wq))
            while dq:
                it = dq.popleft()
                if isinstance(it, tuple):
                    emit_fc1_chunk(0, it[0], 256 * it[1], 256)
                else:
                    emit_fc1_chunk(0, it)

        # attention PSUM pools closed: open a wide tail pool (6 banks)
        with tc.tile_pool(name="ps_tail", bufs=6, space="PSUM") as ps_tail:
            ps_ref[0] = ps_tail
            # ---- tail: ic1 MLP ----
            mu1 = emit_ln2_mu(1, 0, 512)
            for mo in range(DC):
                emit_proj_chunk(1, mo, 0, 512)
            bcs = emit_ln2_s2(1, 0, 256, *mu1)
            bcs_b1 = emit_ln2_s2(1, 0, 512, *mu1)
            emit_ln2_apply(1, bcs, 0, 256, range(DC))
            # band0 gelus go to a buffer aliased on dead v_aug memory, so
            # fc1(ic1) needn't wait for fc2(ic0) to drain gelu_t
            gelu_b0 = mem.tile([P, HC, 256], FP8, tag="vf", name="gelu_b0")
            for mo in range(HC):
                emit_fc1_chunk(1, mo, 0, 256, gdst=gelu_b0[:, mo, :])
                if mo % 2 == 1 and mo < 12:
                    emit_fc2_chunk(0, mo // 2, 0, 512)
                elif mo >= 12 and mo % 3 == 2:
                    emit_exit_tr((mo - 12) // 3)
                if mo == 11:
                    emit_ln2_apply(1, bcs_b1, 256, 256, range(DC))
            for mo in range(HC):
                emit_fc1_chunk(1, mo, 256, 256,
                               gdst=gelu_t[:, mo, 0:256])
                if mo % 4 == 3:
                    emit_fc2_chunk(
                        1, mo // 4, 0, 256,
                        gsrc=lambda kp: gelu_b0[:, 2 * kp:2 * kp + 2, :])
            for t in (4, 5):
                emit_exit_tr(t, tail=True)
            for mo in range(DC):
                emit_fc2_chunk(
                    1, mo, 256, 256,
                    gsrc=lambda kp: gelu_t[:, 2 * kp:2 * kp + 2, 0:256])
            for t in (6, 7):
                emit_exit_tr(t, tail=True)



_NC_CACHE = {}


def _get_nc(qkb_zero=False):
    key = ("nc", qkb_zero)
    if key not in _NC_CACHE:
        _NC_CACHE[key] = build_nc(qkb_zero)
    return _NC_CACHE[key]


def _fold_perm():
    perm = []
    for g in range(3):
        for half in range(2):
            for hh in range(4):
                h = 4 * g + hh
                perm.extend(range(h * 64 + 32 * half, h * 64 + 32 * half + 32))
    return np.asarray(perm)


def _prep_inputs(x, ln1_g, ln1_b, qkv_w, qkv_b, proj_w, proj_b,
                 ln2_g, ln2_b, fc1_w, fc1_b, fc2_w, fc2_b):
    f = lambda a: np.asarray(a, np.float32)
    x = f(x)
    qkv_w, qkv_b = f(qkv_w), f(qkv_b)
    proj_w, proj_b = f(proj_w), f(proj_b)
    fc1_w, fc1_b = f(fc1_w), f(fc1_b)
    fc2_w, fc2_b = f(fc2_w), f(fc2_b)
    ln1_g, ln1_b, ln2_g, ln2_b = f(ln1_g), f(ln1_b), f(ln2_g), f(ln2_b)

    # fold LN affine into the following matmul
    qkv_w_eff = ln1_g[:, None] * qkv_w
    qkv_b_eff = qkv_b + ln1_b @ qkv_w
    fc1_w_eff = ln2_g[:, None] * fc1_w
    fc1_b_eff = (fc1_b + ln2_b @ fc1_w).astype(np.float32)

    # v bias commutes through softmax -> fold into proj bias
    vb = qkv_b_eff[2 * D:]
    proj_b_eff = (proj_b + vb @ proj_w).astype(np.float32)

    # fold permutation for q/k DoubleRow scores
    perm = _fold_perm()
    wq = qkv_w_eff[:, 0:D][:, perm]
    wk = qkv_w_eff[:, D:2 * D][:, perm]
    wv = qkv_w_eff[:, 2 * D:]
    wqkv = np.concatenate([wq, wk, wv], axis=1).astype(FP8_NP)
    qkb = np.concatenate([qkv_b_eff[0:D][perm],
                          qkv_b_eff[D:2 * D][perm]]).astype(np.float32)

    proj_w8 = proj_w.astype(FP8_NP)
    wsum8 = np.repeat(proj_w8.astype(np.float32).sum(axis=1, keepdims=True),
                      32, axis=1).astype(FP8_NP)
    pbsum = proj_b_eff.sum() / D
    biases = np.concatenate([
        qkb, proj_b_eff, fc1_b_eff, fc2_b.astype(np.float32),
        np.full(P, pbsum, np.float32)]).astype(np.float32)
    shared = {
        "wqkv": wqkv, "wsum8": wsum8, "biases": biases,
        "proj_w": proj_w8,
        "fc1_w": fc1_w_eff[:, HID // 2:].astype(BF16_NP),
        "fc1_w8": (16.0 * fc1_w_eff[:, :HID // 2]).astype(FP8_NP),
        "fc2_w": (16.0 * fc2_w).astype(FP8_NP),
    }
    n_cores = x.shape[0]
    return [{"x": np.ascontiguousarray(x[c]).astype(BF16_NP), **shared}
            for c in range(n_cores)]


def kernel(**inputs):
    in_maps = _prep_inputs(**inputs)
    nc = _get_nc(
        qkb_zero=bool(np.all(in_maps[0]["biases"][:2 * D] == 0.0)))
    res = run_bass_kernel_spmd(nc, in_maps, core_ids=list(range(len(in_maps))))
    return np.stack([r["y"] for r in res.results], axis=0)


if __name__ == "__main__":
    import reference
    inputs = {k: np.asarray(v) for k, v in reference.setup_inputs().items()}
    out = kernel(**inputs)
    print("kernel out", out.shape, out.dtype)



# revision 105
# speedup vs baseline: 1.0010x; 1.0010x over previous
"""Trainium2 Bass kernel for a dense transformer block (PreNorm attn + MLP).

Full inputs: x [8, 1024, 768] f32 + LN/attn/MLP weights.
Sharding: pure data-parallel — batch 8 across 8 NeuronCores, no collectives.

Per-core design (tokens n=1024, d=768, heads=12, dh=64, hidden=3072):
  - Residual spine fp32 (x2) / bf16 (x), FEATURE-major; weights [d_in, d_out]
    serve as lhsT directly.
  - Deep matmuls fp8e4 DoubleRow (0.5 cyc/row contracting two 128-k-tiles);
    fc1 is MIXED: hidden 0:1536 fp8 DR (weights x16, gelu scale 1/16),
    1536:3072 bf16 — full-fp8 fc1 would blow the 2e-2 error budget.
  - Scores: q/k "folded" — head h on 32 partitions, dh split into the 2
    DoubleRow k-tiles (host-side qkv_w column permutation makes this free).
  - attnV: lhsT = v_aug [j, 2, 96]: 64 v dims + ones col (softmax
    denominator) + 31 zero pad; v bias folded into proj_b host-side.
    1/denom broadcast via PE ones-matmul + DVE copy (gpsimd can't touch
    PSUM; 0-stride-partition SBUF DMA is rejected).
  - LN1 is TOKEN-major: bn_stats/bn_aggr per 128-token block as x lands,
    h1 = tensor_scalar((x-mu)*rstd) per block, then PE-transposed to
    feature-major (x first — it only needs the DMA; h1 second).  Kills the
    old stats->broadcast->apply chain.  pbsum is folded into muxr.
  - exp on ACT is THE bottleneck (~100us); scheduling keeps ACT hot:
    ic0 MLP (proj/LN2/fc1) drains between score-pairs of ic1's exp stream
    (bf16 fc1 split 256-wide to fit the 1.04us exp cadence); the 12 fp8
    fc1(ic0) chunks + 4 bf16 quarters are emitted in the attnV flush so
    their gelus seamlessly extend the exp stream.
  - Tail: attention PSUM pools close, a 6-buf ps_tail pool opens (psum
    rotation was the pacing limit); LN2 mu from the wsum trick before proj,
    s2/var fused via ACT Square + stt, f32 PE broadcast; band0 (256 tok)
    stats fast-path + full-width pass for band1; band0 gelus into a buffer
    aliased on dead v_aug so fc1(ic1) needn't wait for fc2(ic0); band1 fc1
    interleaves with fc2(ic1) tq0 to keep the gelu stream continuous.
  - DMAs coalesced (26 total): x in 4, each weight matrix 1-2, biases
    packed into one [128,49] tensor host-side.
"""

import collections

import numpy as np
import ml_dtypes

import concourse.bass as bass
import concourse.tile as tile
from concourse import mybir
from concourse.masks import make_identity
from concourse.bass_utils import run_bass_kernel_spmd

F32 = mybir.dt.float32
BF16 = mybir.dt.bfloat16
FP8 = mybir.dt.float8e4
BF16_NP = ml_dtypes.bfloat16
FP8_NP = ml_dtypes.float8_e4m3
AF = mybir.ActivationFunctionType
DR = mybir.MatmulPerfMode.DoubleRow
ADD = mybir.AluOpType.add

N = 1024          # tokens per core
D = 768           # model dim
H = 12            # heads
DH = 64           # head dim
HID = 3072        # mlp hidden
P = 128
NT = N // P       # 8 token chunks
DC = D // P       # 6 feature chunks
HC = HID // P     # 24 hidden chunks
KP = DC // 2      # 3 contraction pairs for d=768
IC = 2            # token halves of 512
VA = 96           # attnV stationary cols: 64 v + ones + 31 pad
EPS = 1e-5


def build_nc(qkb_zero=False):
    nc = bass.Bass("TRN2")

    x_d = nc.dram_tensor("x", [N, D], BF16, kind="ExternalInput")
    wqkv_d = nc.dram_tensor("wqkv", [D, 3 * D], FP8, kind="ExternalInput")
    projw_d = nc.dram_tensor("proj_w", [D, D], FP8, kind="ExternalInput")
    wsum_d = nc.dram_tensor("wsum8", [D, 32], FP8, kind="ExternalInput")
    biases_d = nc.dram_tensor("biases", [49 * P], F32, kind="ExternalInput")
    fc1w_d = nc.dram_tensor("fc1_w", [D, HID // 2], BF16,
                            kind="ExternalInput")
    fc1w8_d = nc.dram_tensor("fc1_w8", [D, HID // 2], FP8,
                             kind="ExternalInput")
    fc2w_d = nc.dram_tensor("fc2_w", [HID, D], FP8, kind="ExternalInput")
    y_d = nc.dram_tensor("y", [N, D], F32, kind="ExternalOutput")

    with tile.TileContext(nc) as tc:
        _body(nc, tc, x_d, wqkv_d, biases_d, projw_d,
              fc1w_d, fc1w8_d, fc2w_d, y_d, qkb_zero, wsum_d)
    # this container's walrus accepts at most 1 sync wait per instruction
    # (2 on EventSemaphore); redistribute excess waits like Bacc.compile does
    import bass_rust as _br
    _br.move_matmul_waits_to_ldweights(nc.m)
    _br.generate_event_semaphores(nc)
    return nc


def _body(nc, tc, x_d, wqkv_d, biases_d, projw_d,
          fc1w_d, fc1w8_d, fc2w_d, y_d, qkb_zero, wsum_d):
    from contextlib import ExitStack
    with ExitStack() as ctx:
        consts = ctx.enter_context(tc.tile_pool(name="consts", bufs=1))
        rows = ctx.enter_context(tc.tile_pool(name="rows", bufs=3))
        recp = ctx.enter_context(tc.tile_pool(name="recp", bufs=2))
        rbp = ctx.enter_context(tc.tile_pool(name="rbp", bufs=2))
        ltp = ctx.enter_context(tc.tile_pool(name="ltp", bufs=2))
        ftp = ctx.enter_context(tc.tile_pool(name="ftp", bufs=1))
        bcp = ctx.enter_context(tc.tile_pool(name="bcp", bufs=2))
        dram = ctx.enter_context(tc.tile_pool(name="dram", bufs=1, space="DRAM"))
        ps_mm = ctx.enter_context(tc.tile_pool(name="ps_mm", bufs=2, space="PSUM"))
        ps_ref = [ps_mm]
        mem = ctx.enter_context(tc.tile_pool(name="mem", bufs=1))
        mem2 = ctx.enter_context(tc.tile_pool(name="mem2", bufs=2))

        ALP = nc.allow_low_precision

        # ---- constants & early DMAs (x first, then q/k weights) ----
        ident = consts.tile([P, P], F32, tag="ident")
        make_identity(nc, ident)
        ident_bf = consts.tile([P, P], BF16, tag="ident_bf")
        make_identity(nc, ident_bf)

        x_tok = mem.tile([P, NT, D], BF16, tag="xo")           # -> out_fm later
        x_view = x_d[:, :].rearrange("(t p) d -> p t d", p=P)
        for tp in range(4):
            nc.sync.dma_start(out=x_tok[:, 2 * tp:2 * tp + 2, :],
                              in_=x_view[:, 2 * tp:2 * tp + 2, :])

        wqkv_sb = mem.tile([P, DC, 3 * D], FP8, tag="w1")      # -> fc1w later
        wqkv_v = wqkv_d[:, :].rearrange("(ko p) m -> p ko m", p=P)
        nc.sync.dma_start(out=wqkv_sb[:, :, 0:2 * D],
                          in_=wqkv_v[:, :, 0:2 * D])

        ones8 = consts.tile([P, 2, 32], FP8, tag="ones8")
        nc.vector.memset(ones8, 1.0)
        onesb_c = consts.tile([P, 1], BF16, tag="onesb_c")     # stats lhsT
        nc.vector.memset(onesb_c, 1.0)
        onesb_r = consts.tile([1, P], BF16, tag="onesb_r")     # bcast lhsT
        nc.vector.memset(onesb_r, 1.0)
        onesb_rf = consts.tile([1, P], F32, tag="onesb_rf")    # f32 bcast
        nc.vector.memset(onesb_rf, 1.0)
        eps_sb = consts.tile([P, 1], F32, tag="eps_sb")
        nc.vector.memset(eps_sb, EPS)

        # biases packed host-side: cols 0:12 qkb, 12:18 projb, 18:42 fc1b,
        # 42:48 fc2b, 48 pbsum (broadcast)
        bias_all = consts.tile([P, 49], F32, tag="biases")
        nc.sync.dma_start(out=bias_all,
                          in_=biases_d[:].rearrange("(mo p) -> p mo", p=P))
        qkb_sb = bias_all[:, 0:12]
        projb_sb = bias_all[:, 12:18]
        fc1b_sb = bias_all[:, 18:42]
        fc2b_sb = bias_all[:, 42:48]
        pbs_sb = bias_all[0:1, 48:49]

        # deferred weight DMAs (after x / qk in the DMA queues)
        nc.sync.dma_start(out=wqkv_sb[:, :, 2 * D:],
                          in_=wqkv_v[:, :, 2 * D:])
        wsum_sb = consts.tile([P, DC, 32], FP8, tag="wsum")
        nc.sync.dma_start(out=wsum_sb,
                          in_=wsum_d[:, :].rearrange("(ko p) m -> p ko m", p=P))
        projw_sb = mem.tile([P, DC, D], FP8, tag="pw")
        projw_v = projw_d[:, :].rearrange("(ko p) m -> p ko m", p=P)
        nc.sync.dma_start(out=projw_sb, in_=projw_v)

        # v_aug: ones col + zero pad (finite garbage would still poison psum)
        v_aug = mem.tile([P, NT, H, VA], FP8, tag="vf")

        # ---- lead-in: token-major LN1 (per-token stats on free axis) ----
        x_fm = mem.tile([P, DC, N], BF16, tag="xf")
        muxr = mem.tile([1, N], BF16, tag="mux")
        h1 = mem.tile([P, DC, N], FP8, tag="ha")               # -> h2 later
        h1_tok = mem.tile([P, NT, D], BF16, tag="ge")      # -> gelu_t later
        mv_tok = mem.tile([P, NT, 2], F32, tag="mvt")
        rstd_tok = mem.tile([P, NT], F32, tag="rst")

        SUB = mybir.AluOpType.subtract
        MUL = mybir.AluOpType.mult
        for t in range(NT):
            bs = rows.tile([P, 2, 6], F32, tag="bs", name="bn")
            nc.vector.bn_stats(bs[:, 0, :], x_tok[:, t, 0:384])
            nc.vector.bn_stats(bs[:, 1, :], x_tok[:, t, 384:768])
            nc.vector.bn_aggr(mv_tok[:, t, :], bs)
            if t % 4 == 3:
                hf = t // 4
                nc.scalar.activation(
                    out=rstd_tok[:, 4 * hf:4 * hf + 4],
                    in_=mv_tok[:, 4 * hf:4 * hf + 4, 1],
                    func=AF.Sqrt, bias=eps_sb, scale=1.0)
                nc.vector.reciprocal(
                    out=rstd_tok[:, 4 * hf:4 * hf + 4],
                    in_=rstd_tok[:, 4 * hf:4 * hf + 4])
                for tt in range(4 * hf, 4 * hf + 4):
                    with ALP(reason="fp8 h1"):
                        nc.vector.tensor_scalar(
                            out=h1_tok[:, tt, :], in0=x_tok[:, tt, :],
                            scalar1=mv_tok[:, tt, 0:1],
                            scalar2=rstd_tok[:, tt:tt + 1],
                            op0=SUB, op1=MUL)

        nc.gpsimd.memset(v_aug[:, :, :, DH + 1:], 0.0)
        nc.gpsimd.memset(v_aug[:, :, :, DH:DH + 1], 1.0)
        # transposes to feature-major: x first (only needs the DMA), h1
        # second (gated by the per-token stats chain)
        with tc.tile_pool(name="ps_lead", bufs=4, space="PSUM") as ps_lead:
            for hf in range(2):
                sl = slice(hf * 512, (hf + 1) * 512)
                for dc in range(DC):
                    pt = ps_lead.tile([P, 4, P], BF16, tag="tr", name="ptx")
                    for q in range(4):
                        t = hf * 4 + q
                        nc.tensor.transpose(
                            pt[:, q, :], x_tok[:, t, dc * P:(dc + 1) * P],
                            ident_bf)
                    with ALP(reason="bf16 x_fm"):
                        nc.scalar.copy(
                            out=x_fm[:, dc, sl],
                            in_=pt.rearrange("p a b -> p (a b)"))
            for hf in range(2):
                sl = slice(hf * 512, (hf + 1) * 512)
                for dc in range(DC):
                    ph = ps_lead.tile([P, 4, P], BF16, tag="tr", name="pth")
                    for q in range(4):
                        t = hf * 4 + q
                        nc.tensor.transpose(
                            ph[:, q, :], h1_tok[:, t, dc * P:(dc + 1) * P],
                            ident_bf)
                    heng = nc.scalar.copy if dc % 2 else nc.vector.tensor_copy
                    with ALP(reason="fp8 h1 fm"):
                        heng(out=h1[:, dc, sl],
                             in_=ph.rearrange("p a b -> p (a b)"))
                    if dc == DC - 1:
                        psm = ps_lead.tile([1, 512], F32, tag="tr",
                                           name="ps_mu1")
                        for k in range(DC):
                            nc.tensor.matmul(psm, onesb_c, x_fm[:, k, sl],
                                             start=(k == 0),
                                             stop=(k == DC - 1))
                        with ALP(reason="bf16 mux"):
                            nc.vector.tensor_scalar(
                                out=muxr[0:1, sl], in0=psm, scalar1=1.0 / D,
                                scalar2=pbs_sb,
                                op0=mybir.AluOpType.mult, op1=ADD)


        # ---- DoubleRow helpers ----
        def dr_group(ps_ap, lhs_fn, rhs_fn, nkp):
            for kp in range(nkp):
                nc.tensor.matmul(ps_ap, lhs_fn(kp), rhs_fn(kp),
                                 start=(kp == 0), stop=(kp == nkp - 1),
                                 perf_mode=DR)

        q_fold = mem.tile([P, 3, 2, N], FP8, tag="qf")
        k_fold = mem.tile([P, 3, 2, N], FP8, tag="kf")

        def emit_qk_chunk(j, ic, act=False):
            """j in 0..11: q chunks 0-5 as (g, half), k chunks 6-11."""
            g, half = divmod(j % 6, 2)
            dst = k_fold if j >= 6 else q_fold
            sl = slice(ic * 512, (ic + 1) * 512)
            ps = ps_ref[0].tile([P, 512], F32, tag="mm", name="ps_qk")
            dr_group(ps,
                     lambda kp: wqkv_sb[:, 2 * kp:2 * kp + 2, j * P:(j + 1) * P],
                     lambda kp: h1[:, 2 * kp:2 * kp + 2, sl], KP)
            with ALP(reason="fp8 qk"):
                if act and qkb_zero:
                    nc.scalar.copy(out=dst[:, g, half, sl], in_=ps)
                elif qkb_zero:
                    nc.vector.tensor_copy(out=dst[:, g, half, sl], in_=ps)
                else:
                    nc.vector.tensor_scalar_add(out=dst[:, g, half, sl],
                                                in0=ps,
                                                scalar1=qkb_sb[:, j:j + 1])

        def emit_v_chunk(t, vc):
            fw = 512 if vc == 0 else 256
            ps = ps_ref[0].tile([P, 512], F32, tag="mm", name="ps_v")
            dr_group(ps[:, :fw],
                     lambda kp: h1[:, 2 * kp:2 * kp + 2, t * P:(t + 1) * P],
                     lambda kp: wqkv_sb[:, 2 * kp:2 * kp + 2,
                                        2 * D + vc * 512:2 * D + vc * 512 + fw],
                     KP)
            with ALP(reason="fp8 v"):
                nc.vector.tensor_copy(
                    out=v_aug[:, t, vc * 8:vc * 8 + fw // DH, 0:DH],
                    in_=ps[:, :fw].rearrange("p (h e) -> p h e", e=DH))

        fc1w_sb = mem.tile([P, DC, HID // 2], BF16, tag="w1")
        fc1w_v = fc1w_d[:, :].rearrange("(ko p) m -> p ko m", p=P)
        fc1w8_sb = mem.tile([P, DC, HID // 2], FP8, tag="w18")
        fc1w8_v = fc1w8_d[:, :].rearrange("(ko p) m -> p ko m", p=P)
        fc2w_sb = mem.tile([P, HC, D], FP8, tag="f2")
        fc2w_v = fc2w_d[:, :].rearrange("(ko p) m -> p ko m", p=P)

        x2_fm = mem.tile([P, DC, N], F32, tag="x2")
        attn_fm = mem.tile([P, DC, N], FP8, tag="at")
        x2s = mem.tile([P, DC, 512], FP8, tag="xq")
        gelu_t = mem.tile([P, HC, 512], FP8, tag="ge")
        out_fm = mem.tile([P, DC, N], F32, tag="xo")

        def emit_proj_chunk(ic, mo, q0, qw):
            """token window [ic*512+q0, +qw); x2c/x2s live at [q0, q0+qw)."""
            sl = slice(ic * 512 + q0, ic * 512 + q0 + qw)
            sq = slice(q0, q0 + qw)
            ps = ps_ref[0].tile([P, 512], F32, tag="mm", name="ps_proj")
            dr_group(ps[:, :qw],
                     lambda kp: projw_sb[:, 2 * kp:2 * kp + 2,
                                         mo * P:(mo + 1) * P],
                     lambda kp: attn_fm[:, 2 * kp:2 * kp + 2, sl], KP)
            nc.vector.scalar_tensor_tensor(
                out=x2_fm[:, mo, sl], in0=ps[:, :qw],
                scalar=projb_sb[:, mo:mo + 1], in1=x_fm[:, mo, sl],
                op0=ADD, op1=ADD)
            with ALP(reason="fp8 stats"):
                enq = nc.gpsimd if mo % 2 else nc.vector
                enq.tensor_mul(x2s[:, mo, sq], x2_fm[:, mo, sl],
                               x2_fm[:, mo, sl])

        def emit_ln2_mu(ic, q0, qw):
            gsl = slice(ic * 512 + q0, ic * 512 + q0 + qw)
            murow = rows.tile([1, 512], F32, tag="row", name="mu2row")
            psr = ps_ref[0].tile([32, 512], F32, tag="mm", name="ps_mu2")
            dr_group(psr[:, :qw], lambda kp: wsum_sb[:, 2 * kp:2 * kp + 2, :],
                     lambda kp: attn_fm[:, 2 * kp:2 * kp + 2, gsl], KP)
            # muxr carries mean_d(x) + pbsum (folded at lead-in)
            nc.vector.scalar_tensor_tensor(
                out=murow[:, :qw], in0=psr[0:1, :qw], scalar=1.0 / D,
                in1=muxr[0:1, gsl], op0=mybir.AluOpType.mult, op1=ADD)
            mu_bf = rows.tile([1, 512], BF16, tag="rowb", name="mu2bf")
            with ALP(reason="bf16 rows"):
                nc.vector.tensor_copy(out=mu_bf[:, :qw], in_=murow[:, :qw])
            mu2_bc = bcp.tile([P, 512], F32, tag="bc", name="mu2_bc")
            psb1 = ps_ref[0].tile([P, 512], F32, tag="mm", name="psb_mu2")
            nc.tensor.matmul(psb1[:, :qw], onesb_r, mu_bf[:, :qw],
                             start=True, stop=True)
            nc.vector.tensor_copy(out=mu2_bc[:, :qw], in_=psb1[:, :qw])
            return murow, mu2_bc

        def emit_ln2_s2(ic, q0, qw, murow, mu2_bc):
            sq = slice(q0, q0 + qw)
            # mu^2 on ACT (idle in the tail window); var fused via stt
            musq = rows.tile([1, 512], F32, tag="row", name="musq")
            nc.scalar.activation(out=musq[:, :qw], in_=murow[:, :qw],
                                 func=AF.Square, scale=1.0)
            pss = ps_ref[0].tile([32, 512], F32, tag="mm", name="ps_s22")
            dr_group(pss[:, :qw], lambda kp: ones8,
                     lambda kp: x2s[:, 2 * kp:2 * kp + 2, sq], KP)
            var = rows.tile([1, 512], F32, tag="row", name="var2")
            nc.vector.scalar_tensor_tensor(
                out=var[:, :qw], in0=pss[0:1, :qw], scalar=1.0 / D,
                in1=musq[:, :qw], op0=mybir.AluOpType.mult,
                op1=mybir.AluOpType.subtract)
            rstd2 = rows.tile([1, 512], F32, tag="row", name="rstd2")
            nc.scalar.activation(out=rstd2[:, :qw], in_=var[:, :qw],
                                 func=AF.Sqrt, bias=eps_sb[0:1, :], scale=1.0)
            nc.vector.reciprocal(out=rstd2[:, :qw], in_=rstd2[:, :qw])
            rstd2_bc = bcp.tile([P, 512], F32, tag="bc", name="rstd2_bc")
            psb2 = ps_ref[0].tile([P, 512], F32, tag="mm", name="psb_rs2")
            nc.tensor.matmul(psb2[:, :qw], onesb_rf, rstd2[:, :qw],
                             start=True, stop=True)
            nc.vector.tensor_copy(out=rstd2_bc[:, :qw], in_=psb2[:, :qw])
            return mu2_bc, rstd2_bc

        def emit_ln2_stats(ic, q0, qw):
            murow, mu2_bc = emit_ln2_mu(ic, q0, qw)
            return emit_ln2_s2(ic, q0, qw, murow, mu2_bc)

        h2 = [None, None]
        h2_8 = [None, None]

        def emit_ln2_apply(ic, bcs, q0, qw, dcs):
            mu2_bc, rstd2_bc = bcs
            sl = slice(ic * 512 + q0, ic * 512 + q0 + qw)
            sq = slice(q0, q0 + qw)
            if h2[ic] is None:
                h2[ic] = mem.tile([P, DC, 512], BF16, tag="ha", name=f"h2_{ic}")
                h2_8[ic] = mem.tile([P, DC, 512], FP8, tag="h28",
                                    name=f"h28_{ic}")
            for dc in dcs:
                engA, engB = ((nc.vector, nc.gpsimd) if dc % 2 == 0
                              else (nc.gpsimd, nc.vector))
                lt = ltp.tile([P, 512], BF16, tag="lt", name="ln2_tmp")
                with ALP(reason="ln2"):
                    engA.tensor_sub(lt[:, :qw], x2_fm[:, dc, sl],
                                    mu2_bc[:, q0:q0 + qw])
                    engA.tensor_mul(h2[ic][:, dc, sq], lt[:, :qw],
                                    rstd2_bc[:, q0:q0 + qw])
                    engB.tensor_mul(h2_8[ic][:, dc, sq], lt[:, :qw],
                                    rstd2_bc[:, q0:q0 + qw])

        def emit_fc1_chunk(ic, mo, q0=0, qw=512, gdst=None):
            sq = slice(q0, q0 + qw)
            ps = ps_ref[0].tile([P, 512], F32, tag="mm", name="ps_fc1")
            if mo < HC // 2:
                dr_group(ps[:, :qw],
                         lambda kp: fc1w8_sb[:, 2 * kp:2 * kp + 2,
                                             mo * P:(mo + 1) * P],
                         lambda kp: h2_8[ic][:, 2 * kp:2 * kp + 2, sq], KP)
                scale = 1.0 / 16.0
            else:
                mb = mo - HC // 2
                for k in range(DC):
                    nc.tensor.matmul(ps[:, :qw],
                                     fc1w_sb[:, k, mb * P:(mb + 1) * P],
                                     h2[ic][:, k, sq],
                                     start=(k == 0), stop=(k == DC - 1))
                scale = 1.0
            if gdst is None:
                gdst = gelu_t[:, mo, sq]
            with ALP(reason="fp8 gelu"):
                nc.scalar.activation(out=gdst, in_=ps[:, :qw],
                                     func=AF.Gelu,
                                     bias=fc1b_sb[:, mo:mo + 1], scale=scale)

        def emit_fc2_chunk(ic, mo, q0, qw, gsrc=None):
            sl = slice(ic * 512 + q0, ic * 512 + q0 + qw)
            sq = slice(q0, q0 + qw)
            if gsrc is None:
                gsrc = lambda kp: gelu_t[:, 2 * kp:2 * kp + 2, sq]
            ps = ps_ref[0].tile([P, 512], F32, tag="mm", name="ps_fc2")
            dr_group(ps[:, :qw],
                     lambda kp: fc2w_sb[:, 2 * kp:2 * kp + 2,
                                        mo * P:(mo + 1) * P],
                     gsrc, HC // 2)
            ft = ftp.tile([P, 512], BF16, tag="ft", name="fc2_tmp")
            with ALP(reason="bf16 fc2 tmp"):
                nc.vector.tensor_scalar(out=ft[:, :qw], in0=ps[:, :qw],
                                        scalar1=1.0 / 16.0,
                                        scalar2=fc2b_sb[:, mo:mo + 1],
                                        op0=mybir.AluOpType.mult, op1=ADD)
            nc.gpsimd.tensor_add(out_fm[:, mo, sl], ft[:, :qw],
                                 x2_fm[:, mo, sl])

        def emit_exit_tr(t, tail=False):
            y_stage = mem2.tile([P, D], F32, tag="ys", name="y_stage")
            for dg in range(2):
                pt = ps_ref[0].tile([P, 3, P], F32, tag="mm", name="ps_tr2")
                for q in range(3):
                    dc = dg * 3 + q
                    nc.tensor.transpose(pt[:, q, :],
                                        out_fm[:, dc, t * P:(t + 1) * P],
                                        ident)
                eng = nc.scalar.copy if tail and dg % 2 \
                    else nc.vector.tensor_copy
                eng(out=y_stage[:, dg * 3 * P:(dg + 1) * 3 * P],
                    in_=pt.rearrange("p a b -> p (a b)"))
            nc.sync.dma_start(out=y_d[t * P:(t + 1) * P, :], in_=y_stage)

        # ---- attention + work-queue schedule ----
        wq = collections.deque()

        def drain(n):
            for _ in range(min(n, len(wq))):
                wq.popleft()()

        def refill(ic, h):
            if ic == 0:
                if h == 0:
                    for t in range(NT):
                        for vc in range(2):
                            wq.append(lambda t=t, vc=vc: emit_v_chunk(t, vc))
                elif h == 1:
                    for j in (2, 3, 8, 9):
                        for i2 in range(IC):
                            wq.append(lambda j=j, i2=i2: emit_qk_chunk(j, i2))
                elif h == 2:
                    for j in (4, 5, 10, 11):
                        for i2 in range(IC):
                            wq.append(lambda j=j, i2=i2: emit_qk_chunk(j, i2))
                if h == 7:
                    nc.sync.dma_start(out=fc1w8_sb, in_=fc1w8_v)
                elif h in (8, 9):
                    ko = 3 * (h - 8)
                    nc.sync.dma_start(out=fc1w_sb[:, ko:ko + 3, :],
                                      in_=fc1w_v[:, ko:ko + 3, :])
                elif h == 11:
                    nc.sync.dma_start(out=fc2w_sb[:, 0:4, :],
                                      in_=fc2w_v[:, 0:4, :])
            else:
                if h < 5:
                    ko = 4 * h + 4
                    nc.sync.dma_start(out=fc2w_sb[:, ko:ko + 4, :],
                                      in_=fc2w_v[:, ko:ko + 4, :])
                if h == 1:
                    # attn_fm(ic0) complete once attnV(h11, ic0) drained (h0)
                    for mo in range(DC):
                        wq.append(lambda mo=mo: emit_proj_chunk(0, mo, 0, 512))
                elif h == 2:
                    def stats0():
                        _st["bcs0"] = emit_ln2_stats(0, 0, 512)
                    wq.append(stats0)
                    wq.append(lambda: emit_ln2_apply(0, _st["bcs0"], 0, 512,
                                                     range(3)))
                    wq.append(lambda: emit_ln2_apply(0, _st["bcs0"], 0, 512,
                                                     range(3, DC)))
                elif h in (3, 4, 5, 6, 7, 8):
                    hi = 12 + 2 * (h - 3)
                    for mo in range(hi, min(hi + 2, 22)):
                        for q in range(2):
                            wq.append(lambda mo=mo, q=q:
                                      emit_fc1_chunk(0, mo, 256 * q, 256))

        def emit_attnv(h, ic, expT):
            pso = ps_att.tile([VA, 512], F32, tag="att", name="pso")
            for c in range(NT // 2):
                nc.tensor.matmul(pso, v_aug[:, 2 * c:2 * c + 2, h, :],
                                 expT[:, 2 * c:2 * c + 2, :],
                                 start=(c == 0), stop=(c == NT // 2 - 1),
                                 perf_mode=DR)
            rec = recp.tile([1, 512], BF16, tag="rec", name="rec")
            with ALP(reason="bf16 recip"):
                nc.vector.reciprocal(out=rec, in_=pso[DH:DH + 1, :])
            # PE broadcast of 1/denom into PSUM, DVE copy to SBUF
            psb = ps_ref[0].tile([P, 512], F32, tag="mm", name="psb")
            nc.tensor.matmul(psb[0:DH, :], onesb_r[:, 0:DH], rec,
                             start=True, stop=True)
            rb = rbp.tile([DH, 512], BF16, tag="rb", name="rb")
            with ALP(reason="bf16 rb"):
                nc.vector.tensor_copy(out=rb, in_=psb[0:DH, :])
            with ALP(reason="fp8 attn"):
                nc.vector.tensor_mul(
                    out=attn_fm[64 * (h % 2):64 * (h % 2) + 64,
                                h // 2, ic * 512:(ic + 1) * 512],
                    in0=pso[0:DH, :], in1=rb)

        _st = {}
        DEPTH = 1  # attnV(h) emitted after scores(h+DEPTH)
        with tc.tile_pool(name="ps_sc", bufs=2, space="PSUM") as ps_sc, \
             tc.tile_pool(name="ps_att", bufs=2, space="PSUM") as ps_att, \
             tc.tile_pool(name="expp", bufs=DEPTH + 1) as expp:
            for j in (6, 7, 0, 1):
                for i2 in range(IC):
                    emit_qk_chunk(j, i2, act=(j >= 6))
            pend = collections.deque()
            for ic in range(IC):
                for h in range(H):
                    refill(ic, h)
                    g, b = divmod(h, 4)
                    p0 = 32 * b
                    expT = expp.tile([P, NT, 512], FP8, tag="ex", name="expT")
                    for jp in range(NT // 2):
                        ps = ps_sc.tile([P, 2, 512], F32, tag="sc",
                                        name="ps_sc")
                        for half in range(2):
                            jc = 2 * jp + half
                            nc.tensor.matmul(
                                ps[:, half, :],
                                k_fold[p0:p0 + 32, g, :, jc * P:(jc + 1) * P],
                                q_fold[p0:p0 + 32, g, :,
                                       ic * 512:(ic + 1) * 512],
                                start=True, stop=True, perf_mode=DR,
                                tile_position=(p0, 0))
                        with ALP(reason="fp8 exp"):
                            nc.scalar.activation(
                                out=expT[:, 2 * jp:2 * jp + 2, :], in_=ps,
                                func=AF.Exp, scale=0.125)
                        drain(2 if h < 5 else 1)
                    pend.append((h, ic, expT))
                    if len(pend) > DEPTH:
                        emit_attnv(*pend.popleft())
                    drain(2)
            dq = collections.deque(
                list(range(12)) + [(22, 0), (22, 1), (23, 0), (23, 1)])
            while pend:
                emit_attnv(*pend.popleft())
                drain(2)
                for _ in range(4):
                    if dq:
                        it = dq.popleft()
                        if isinstance(it, tuple):
                            emit_fc1_chunk(0, it[0], 256 * it[1], 256)
                        else:
                            emit_fc1_chunk(0, it)
            drain(len(wq))
            while dq:
                it = dq.popleft()
                if isinstance(it, tuple):
                    emit_fc1_chunk(0, it[0], 256 * it[1], 256)
                else:
                    emit_fc1_chunk(0, it)

        # attention PSUM pools closed: open a wide tail pool (6 banks)
        with tc.tile_pool(name="ps_tail", bufs=6, space="PSUM") as ps_tail:
            ps_ref[0] = ps_tail
            # ---- tail: ic1 MLP ----
            mu1 = emit_ln2_mu(1, 0, 512)
            for mo in range(DC):
                emit_proj_chunk(1, mo, 0, 512)
            bcs = emit_ln2_s2(1, 0, 256, *mu1)
            bcs_b1 = emit_ln2_s2(1, 0, 512, *mu1)
            emit_ln2_apply(1, bcs, 0, 256, range(DC))
            # band0 gelus go to a buffer aliased on dead v_aug memory, so
            # fc1(ic1) needn't wait for fc2(ic0) to drain gelu_t
            gelu_b0 = mem.tile([P, HC, 256], FP8, tag="vf", name="gelu_b0")
            for mo in range(HC):
                emit_fc1_chunk(1, mo, 0, 256, gdst=gelu_b0[:, mo, :])
                if mo % 2 == 1 and mo < 12:
                    emit_fc2_chunk(0, mo // 2, 0, 512)
                elif mo >= 12 and mo % 3 == 2:
                    emit_exit_tr((mo - 12) // 3)
                if mo == 11:
                    emit_ln2_apply(1, bcs_b1, 256, 256, range(DC))
            for mo in range(HC):
                emit_fc1_chunk(1, mo, 256, 256,
                               gdst=gelu_t[:, mo, 0:256])
                if mo % 4 == 3:
                    emit_fc2_chunk(
                        1, mo // 4, 0, 256,
                        gsrc=lambda kp: gelu_b0[:, 2 * kp:2 * kp + 2, :])
            for t in (4, 5):
                emit_exit_tr(t, tail=True)
            for mo in range(DC):
                emit_fc2_chunk(
                    1, mo, 256, 256,
                    gsrc=lambda kp: gelu_t[:, 2 * kp:2 * kp + 2, 0:256])
            for t in (6, 7):
                emit_exit_tr(t, tail=True)



_NC_CACHE = {}


def _get_nc(qkb_zero=False):
    key = ("nc", qkb_zero)
    if key not in _NC_CACHE:
        _NC_CACHE[key] = build_nc(qkb_zero)
    return _NC_CACHE[key]


def _fold_perm():
    perm = []
    for g in range(3):
        for half in range(2):
            for hh in range(4):
                h = 4 * g + hh
                perm.extend(range(h * 64 + 32 * half, h * 64 + 32 * half + 32))
    return np.asarray(perm)


def _prep_inputs(x, ln1_g, ln1_b, qkv_w, qkv_b, proj_w, proj_b,
                 ln2_g, ln2_b, fc1_w, fc1_b, fc2_w, fc2_b):
    f = lambda a: np.asarray(a, np.float32)
    x = f(x)
    qkv_w, qkv_b = f(qkv_w), f(qkv_b)
    proj_w, proj_b = f(proj_w), f(proj_b)
    fc1_w, fc1_b = f(fc1_w), f(fc1_b)
    fc2_w, fc2_b = f(fc2_w), f(fc2_b)
    ln1_g, ln1_b, ln2_g, ln2_b = f(ln1_g), f(ln1_b), f(ln2_g), f(ln2_b)

    # fold LN affine into the following matmul
    qkv_w_eff = ln1_g[:, None] * qkv_w
    qkv_b_eff = qkv_b + ln1_b @ qkv_w
    fc1_w_eff = ln2_g[:, None] * fc1_w
    fc1_b_eff = (fc1_b + ln2_b @ fc1_w).astype(np.float32)

    # v bias commutes through softmax -> fold into proj bias
    vb = qkv_b_eff[2 * D:]
    proj_b_eff = (proj_b + vb @ proj_w).astype(np.float32)

    # fold permutation for q/k DoubleRow scores
    perm = _fold_perm()
    wq = qkv_w_eff[:, 0:D][:, perm]
    wk = qkv_w_eff[:, D:2 * D][:, perm]
    wv = qkv_w_eff[:, 2 * D:]
    wqkv = np.concatenate([wq, wk, wv], axis=1).astype(FP8_NP)
    qkb = np.concatenate([qkv_b_eff[0:D][perm],
                          qkv_b_eff[D:2 * D][perm]]).astype(np.float32)

    proj_w8 = proj_w.astype(FP8_NP)
    wsum8 = np.repeat(proj_w8.astype(np.float32).sum(axis=1, keepdims=True),
                      32, axis=1).astype(FP8_NP)
    pbsum = proj_b_eff.sum() / D
    biases = np.concatenate([
        qkb, proj_b_eff, fc1_b_eff, fc2_b.astype(np.float32),
        np.full(P, pbsum, np.float32)]).astype(np.float32)
    shared = {
        "wqkv": wqkv, "wsum8": wsum8, "biases": biases,
        "proj_w": proj_w8,
        "fc1_w": fc1_w_eff[:, HID // 2:].astype(BF16_NP),
        "fc1_w8": (16.0 * fc1_w_eff[:, :HID // 2]).astype(FP8_NP),
        "fc2_w": (16.0 * fc2_w).astype(FP8_NP),
    }
    n_cores = x.shape[0]
    return [{"x": np.ascontiguousarray(x[c]).astype(BF16_NP), **shared}
            for c in range(n_cores)]


def kernel(**inputs):
    in_maps = _prep_inputs(**inputs)
    nc = _get_nc(
        qkb_zero=bool(np.all(in_maps[0]["biases"][:2 * D] == 0.0)))
    res = run_bass_kernel_spmd(nc, in_maps, core_ids=list(range(len(in_maps))))
    return np.stack([r["y"] for r in res.results], axis=0)


if __name__ == "__main__":
    import reference
    inputs = {k: np.asarray(v) for k, v in reference.setup_inputs().items()}
    out = kernel(**inputs)
    print("kernel out", out.shape, out.dtype)



# revision 111
# speedup vs baseline: 1.0264x; 1.0254x over previous
"""Trainium2 Bass kernel for a dense transformer block (PreNorm attn + MLP).

Full inputs: x [8, 1024, 768] f32 + LN/attn/MLP weights.
Sharding: pure data-parallel — batch 8 across 8 NeuronCores, no collectives.

Per-core design (tokens n=1024, d=768, heads=12, dh=64, hidden=3072):
  - Residual spine fp32 (x2) / bf16 (x), FEATURE-major; weights [d_in, d_out]
    serve as lhsT directly.
  - Deep matmuls fp8e4 DoubleRow (0.5 cyc/row contracting two 128-k-tiles);
    fc1 is MIXED: hidden 0:1536 fp8 DR (weights x16, gelu scale 1/16),
    1536:3072 bf16 — full-fp8 fc1 would blow the 2e-2 error budget.
  - Scores: q/k "folded" — head h on 32 partitions, dh split into the 2
    DoubleRow k-tiles (host-side qkv_w column permutation makes this free).
  - attnV: lhsT = v_aug [j, 2, 96]: 64 v dims + ones col (softmax
    denominator) + 31 zero pad; v bias folded into proj_b host-side.
    1/denom broadcast via PE ones-matmul + DVE copy (gpsimd can't touch
    PSUM; 0-stride-partition SBUF DMA is rejected).
  - LN1 is TOKEN-major: bn_stats/bn_aggr per 128-token block as x lands,
    h1 = tensor_scalar((x-mu)*rstd) per block, then PE-transposed to
    feature-major (x first — it only needs the DMA; h1 second).  Kills the
    old stats->broadcast->apply chain.  pbsum is folded into muxr.
  - exp on ACT is THE bottleneck (~100us); scheduling keeps ACT hot:
    ic0 MLP (proj/LN2/fc1) drains between score-pairs of ic1's exp stream
    (bf16 fc1 split 256-wide to fit the 1.04us exp cadence); the 12 fp8
    fc1(ic0) chunks + 4 bf16 quarters are emitted in the attnV flush so
    their gelus seamlessly extend the exp stream.
  - Tail: attention PSUM pools close, a 6-buf ps_tail pool opens (psum
    rotation was the pacing limit); LN2 mu from the wsum trick before proj,
    s2/var fused via ACT Square + stt, f32 PE broadcast; band0 (256 tok)
    stats fast-path + full-width pass for band1; band0 gelus into a buffer
    aliased on dead v_aug so fc1(ic1) needn't wait for fc2(ic0); band1 fc1
    interleaves with fc2(ic1) tq0 to keep the gelu stream continuous.
  - DMAs coalesced (26 total): x in 4, each weight matrix 1-2, biases
    packed into one [128,49] tensor host-side.
"""

import collections

import numpy as np
import ml_dtypes

import concourse.bass as bass
import concourse.tile as tile
from concourse import mybir
from concourse.masks import make_identity
from concourse.bass_utils import run_bass_kernel_spmd

F32 = mybir.dt.float32
BF16 = mybir.dt.bfloat16
FP8 = mybir.dt.float8e4
BF16_NP = ml_dtypes.bfloat16
FP8_NP = ml_dtypes.float8_e4m3
AF = mybir.ActivationFunctionType
DR = mybir.MatmulPerfMode.DoubleRow
ADD = mybir.AluOpType.add

N = 1024          # tokens per core
D = 768           # model dim
H = 12            # heads
DH = 64           # head dim
HID = 3072        # mlp hidden
P = 128
NT = N // P       # 8 token chunks
DC = D // P       # 6 feature chunks
HC = HID // P     # 24 hidden chunks
KP = DC // 2      # 3 contraction pairs for d=768
IC = 2            # token halves of 512
VA = 96           # attnV stationary cols: 64 v + ones + 31 pad
EPS = 1e-5


def build_nc(qkb_zero=False):
    nc = bass.Bass("TRN2")

    x_d = nc.dram_tensor("x", [N, D], BF16, kind="ExternalInput")
    wqkv_d = nc.dram_tensor("wqkv", [D, 3 * D], FP8, kind="ExternalInput")
    projw_d = nc.dram_tensor("proj_w", [D, D], FP8, kind="ExternalInput")
    wsum_d = nc.dram_tensor("wsum8", [D, 32], FP8, kind="ExternalInput")
    biases_d = nc.dram_tensor("biases", [49 * P], F32, kind="ExternalInput")
    fc1w_d = nc.dram_tensor("fc1_w", [D, HID // 2], BF16,
                            kind="ExternalInput")
    fc1w8_d = nc.dram_tensor("fc1_w8", [D, HID // 2], FP8,
                             kind="ExternalInput")
    fc2w_d = nc.dram_tensor("fc2_w", [HID, D], FP8, kind="ExternalInput")
    y_d = nc.dram_tensor("y", [N, D], F32, kind="ExternalOutput")

    with tile.TileContext(nc) as tc:
        _body(nc, tc, x_d, wqkv_d, biases_d, projw_d,
              fc1w_d, fc1w8_d, fc2w_d, y_d, qkb_zero, wsum_d)
    # this container's walrus accepts at most 1 sync wait per instruction
    # (2 on EventSemaphore); redistribute excess waits like Bacc.compile does
    import bass_rust as _br
    _br.move_matmul_waits_to_ldweights(nc.m)
    _br.generate_event_semaphores(nc)
    return nc


def _body(nc, tc, x_d, wqkv_d, biases_d, projw_d,
          fc1w_d, fc1w8_d, fc2w_d, y_d, qkb_zero, wsum_d):
    from contextlib import ExitStack
    with ExitStack() as ctx:
        consts = ctx.enter_context(tc.tile_pool(name="consts", bufs=1))
        rows = ctx.enter_context(tc.tile_pool(name="rows", bufs=3))
        recp = ctx.enter_context(tc.tile_pool(name="recp", bufs=3))
        rbp = ctx.enter_context(tc.tile_pool(name="rbp", bufs=3))
        ltp = ctx.enter_context(tc.tile_pool(name="ltp", bufs=4))
        ftp = ctx.enter_context(tc.tile_pool(name="ftp", bufs=3))
        bcp = ctx.enter_context(tc.tile_pool(name="bcp", bufs=3))
        dram = ctx.enter_context(tc.tile_pool(name="dram", bufs=1, space="DRAM"))
        ps_mm = ctx.enter_context(tc.tile_pool(name="ps_mm", bufs=2, space="PSUM"))
        ps_ref = [ps_mm]
        mem = ctx.enter_context(tc.tile_pool(name="mem", bufs=1))
        mem2 = ctx.enter_context(tc.tile_pool(name="mem2", bufs=2))

        ALP = nc.allow_low_precision

        # ---- constants & early DMAs (x first, then q/k weights) ----
        ident = consts.tile([P, P], F32, tag="ident")
        make_identity(nc, ident)
        ident_bf = consts.tile([P, P], BF16, tag="ident_bf")
        make_identity(nc, ident_bf)

        x_tok = mem.tile([P, NT, D], BF16, tag="xo")           # -> out_fm later
        x_view = x_d[:, :].rearrange("(t p) d -> p t d", p=P)
        for tp in range(4):
            nc.sync.dma_start(out=x_tok[:, 2 * tp:2 * tp + 2, :],
                              in_=x_view[:, 2 * tp:2 * tp + 2, :])

        wqkv_sb = mem.tile([P, DC, 3 * D], FP8, tag="w1")      # -> fc1w later
        wqkv_v = wqkv_d[:, :].rearrange("(ko p) m -> p ko m", p=P)
        nc.sync.dma_start(out=wqkv_sb[:, :, 0:2 * D],
                          in_=wqkv_v[:, :, 0:2 * D])

        ones8 = consts.tile([P, 2, 32], FP8, tag="ones8")
        nc.vector.memset(ones8, 1.0)
        onesb_c = consts.tile([P, 1], BF16, tag="onesb_c")     # stats lhsT
        nc.vector.memset(onesb_c, 1.0)
        onesb_r = consts.tile([1, P], BF16, tag="onesb_r")     # bcast lhsT
        nc.vector.memset(onesb_r, 1.0)
        onesb_rf = consts.tile([1, P], F32, tag="onesb_rf")    # f32 bcast
        nc.vector.memset(onesb_rf, 1.0)
        eps_sb = consts.tile([P, 1], F32, tag="eps_sb")
        nc.vector.memset(eps_sb, EPS)

        # biases packed host-side: cols 0:12 qkb, 12:18 projb, 18:42 fc1b,
        # 42:48 fc2b, 48 pbsum (broadcast)
        bias_all = consts.tile([P, 49], F32, tag="biases")
        nc.sync.dma_start(out=bias_all,
                          in_=biases_d[:].rearrange("(mo p) -> p mo", p=P))
        qkb_sb = bias_all[:, 0:12]
        projb_sb = bias_all[:, 12:18]
        fc1b_sb = bias_all[:, 18:42]
        fc2b_sb = bias_all[:, 42:48]
        pbs_sb = bias_all[0:1, 48:49]

        # deferred weight DMAs (after x / qk in the DMA queues)
        nc.sync.dma_start(out=wqkv_sb[:, :, 2 * D:],
                          in_=wqkv_v[:, :, 2 * D:])
        wsum_sb = consts.tile([P, DC, 32], FP8, tag="wsum")
        nc.sync.dma_start(out=wsum_sb,
                          in_=wsum_d[:, :].rearrange("(ko p) m -> p ko m", p=P))
        projw_sb = mem.tile([P, DC, D], FP8, tag="pw")
        projw_v = projw_d[:, :].rearrange("(ko p) m -> p ko m", p=P)
        nc.sync.dma_start(out=projw_sb, in_=projw_v)

        # v_aug: ones col + zero pad (finite garbage would still poison psum)
        v_aug = mem.tile([P, NT, H, VA], FP8, tag="vf")

        # ---- lead-in: token-major LN1 (per-token stats on free axis) ----
        x_fm = mem.tile([P, DC, N], BF16, tag="xf")
        muxr = mem.tile([1, N], BF16, tag="mux")
        h1 = mem.tile([P, DC, N], FP8, tag="ha")               # -> h2 later
        h1_tok = mem.tile([P, NT, D], BF16, tag="ge")      # -> gelu_t later
        mv_tok = mem.tile([P, NT, 2], F32, tag="mvt")
        rstd_tok = mem.tile([P, NT], F32, tag="rst")

        SUB = mybir.AluOpType.subtract
        MUL = mybir.AluOpType.mult
        for t in range(NT):
            bs = rows.tile([P, 2, 6], F32, tag="bs", name="bn")
            nc.vector.bn_stats(bs[:, 0, :], x_tok[:, t, 0:384])
            nc.vector.bn_stats(bs[:, 1, :], x_tok[:, t, 384:768])
            nc.vector.bn_aggr(mv_tok[:, t, :], bs)
            if t % 4 == 3:
                hf = t // 4
                nc.scalar.activation(
                    out=rstd_tok[:, 4 * hf:4 * hf + 4],
                    in_=mv_tok[:, 4 * hf:4 * hf + 4, 1],
                    func=AF.Sqrt, bias=eps_sb, scale=1.0)
                nc.vector.reciprocal(
                    out=rstd_tok[:, 4 * hf:4 * hf + 4],
                    in_=rstd_tok[:, 4 * hf:4 * hf + 4])
                for tt in range(4 * hf, 4 * hf + 4):
                    with ALP(reason="fp8 h1"):
                        nc.vector.tensor_scalar(
                            out=h1_tok[:, tt, :], in0=x_tok[:, tt, :],
                            scalar1=mv_tok[:, tt, 0:1],
                            scalar2=rstd_tok[:, tt:tt + 1],
                            op0=SUB, op1=MUL)

        nc.gpsimd.memset(v_aug[:, :, :, DH + 1:], 0.0)
        nc.gpsimd.memset(v_aug[:, :, :, DH:DH + 1], 1.0)
        # transposes to feature-major: x first (only needs the DMA), h1
        # second (gated by the per-token stats chain)
        with tc.tile_pool(name="ps_lead", bufs=4, space="PSUM") as ps_lead:
            for hf in range(2):
                sl = slice(hf * 512, (hf + 1) * 512)
                for dc in range(DC):
                    pt = ps_lead.tile([P, 4, P], BF16, tag="tr", name="ptx")
                    for q in range(4):
                        t = hf * 4 + q
                        nc.tensor.transpose(
                            pt[:, q, :], x_tok[:, t, dc * P:(dc + 1) * P],
                            ident_bf)
                    with ALP(reason="bf16 x_fm"):
                        nc.scalar.copy(
                            out=x_fm[:, dc, sl],
                            in_=pt.rearrange("p a b -> p (a b)"))
            for hf in range(2):
                sl = slice(hf * 512, (hf + 1) * 512)
                for dc in range(DC):
                    ph = ps_lead.tile([P, 4, P], BF16, tag="tr", name="pth")
                    for q in range(4):
                        t = hf * 4 + q
                        nc.tensor.transpose(
                            ph[:, q, :], h1_tok[:, t, dc * P:(dc + 1) * P],
                            ident_bf)
                    heng = nc.scalar.copy if dc % 2 else nc.vector.tensor_copy
                    with ALP(reason="fp8 h1 fm"):
                        heng(out=h1[:, dc, sl],
                             in_=ph.rearrange("p a b -> p (a b)"))
                    if dc == DC - 1:
                        psm = ps_lead.tile([1, 512], F32, tag="tr",
                                           name="ps_mu1")
                        for k in range(DC):
                            nc.tensor.matmul(psm, onesb_c, x_fm[:, k, sl],
                                             start=(k == 0),
                                             stop=(k == DC - 1))
                        with ALP(reason="bf16 mux"):
                            nc.vector.tensor_scalar(
                                out=muxr[0:1, sl], in0=psm, scalar1=1.0 / D,
                                scalar2=pbs_sb,
                                op0=mybir.AluOpType.mult, op1=ADD)


        # ---- DoubleRow helpers ----
        def dr_group(ps_ap, lhs_fn, rhs_fn, nkp):
            for kp in range(nkp):
                nc.tensor.matmul(ps_ap, lhs_fn(kp), rhs_fn(kp),
                                 start=(kp == 0), stop=(kp == nkp - 1),
                                 perf_mode=DR)

        q_fold = mem.tile([P, 3, 2, N], FP8, tag="qf")
        k_fold = mem.tile([P, 3, 2, N], FP8, tag="kf")

        def emit_qk_chunk(j, ic, act=False):
            """j in 0..11: q chunks 0-5 as (g, half), k chunks 6-11."""
            g, half = divmod(j % 6, 2)
            dst = k_fold if j >= 6 else q_fold
            sl = slice(ic * 512, (ic + 1) * 512)
            ps = ps_ref[0].tile([P, 512], F32, tag="mm", name="ps_qk")
            dr_group(ps,
                     lambda kp: wqkv_sb[:, 2 * kp:2 * kp + 2, j * P:(j + 1) * P],
                     lambda kp: h1[:, 2 * kp:2 * kp + 2, sl], KP)
            with ALP(reason="fp8 qk"):
                if act and qkb_zero:
                    nc.scalar.copy(out=dst[:, g, half, sl], in_=ps)
                elif qkb_zero:
                    nc.vector.tensor_copy(out=dst[:, g, half, sl], in_=ps)
                else:
                    nc.vector.tensor_scalar_add(out=dst[:, g, half, sl],
                                                in0=ps,
                                                scalar1=qkb_sb[:, j:j + 1])

        def emit_v_chunk(t, vc):
            fw = 512 if vc == 0 else 256
            ps = ps_ref[0].tile([P, 512], F32, tag="mm", name="ps_v")
            dr_group(ps[:, :fw],
                     lambda kp: h1[:, 2 * kp:2 * kp + 2, t * P:(t + 1) * P],
                     lambda kp: wqkv_sb[:, 2 * kp:2 * kp + 2,
                                        2 * D + vc * 512:2 * D + vc * 512 + fw],
                     KP)
            with ALP(reason="fp8 v"):
                nc.vector.tensor_copy(
                    out=v_aug[:, t, vc * 8:vc * 8 + fw // DH, 0:DH],
                    in_=ps[:, :fw].rearrange("p (h e) -> p h e", e=DH))

        fc1w_sb = mem.tile([P, DC, HID // 2], BF16, tag="w1")
        fc1w_v = fc1w_d[:, :].rearrange("(ko p) m -> p ko m", p=P)
        fc1w8_sb = mem.tile([P, DC, HID // 2], FP8, tag="w18")
        fc1w8_v = fc1w8_d[:, :].rearrange("(ko p) m -> p ko m", p=P)
        fc2w_sb = mem.tile([P, HC, D], FP8, tag="f2")
        fc2w_v = fc2w_d[:, :].rearrange("(ko p) m -> p ko m", p=P)

        x2_fm = mem.tile([P, DC, N], F32, tag="x2")
        attn_fm = mem.tile([P, DC, N], FP8, tag="at")
        x2s = mem.tile([P, DC, 512], FP8, tag="xq")
        gelu_t = mem.tile([P, HC, 512], FP8, tag="ge")
        out_fm = mem.tile([P, DC, N], F32, tag="xo")

        def emit_proj_chunk(ic, mo, q0, qw):
            """token window [ic*512+q0, +qw); x2c/x2s live at [q0, q0+qw)."""
            sl = slice(ic * 512 + q0, ic * 512 + q0 + qw)
            sq = slice(q0, q0 + qw)
            ps = ps_ref[0].tile([P, 512], F32, tag="mm", name="ps_proj")
            dr_group(ps[:, :qw],
                     lambda kp: projw_sb[:, 2 * kp:2 * kp + 2,
                                         mo * P:(mo + 1) * P],
                     lambda kp: attn_fm[:, 2 * kp:2 * kp + 2, sl], KP)
            nc.vector.scalar_tensor_tensor(
                out=x2_fm[:, mo, sl], in0=ps[:, :qw],
                scalar=projb_sb[:, mo:mo + 1], in1=x_fm[:, mo, sl],
                op0=ADD, op1=ADD)
            with ALP(reason="fp8 stats"):
                enq = nc.gpsimd if mo % 2 else nc.vector
                enq.tensor_mul(x2s[:, mo, sq], x2_fm[:, mo, sl],
                               x2_fm[:, mo, sl])

        def emit_ln2_mu(ic, q0, qw):
            gsl = slice(ic * 512 + q0, ic * 512 + q0 + qw)
            murow = rows.tile([1, 512], F32, tag="row", name="mu2row")
            psr = ps_ref[0].tile([32, 512], F32, tag="mm", name="ps_mu2")
            dr_group(psr[:, :qw], lambda kp: wsum_sb[:, 2 * kp:2 * kp + 2, :],
                     lambda kp: attn_fm[:, 2 * kp:2 * kp + 2, gsl], KP)
            # muxr carries mean_d(x) + pbsum (folded at lead-in)
            nc.vector.scalar_tensor_tensor(
                out=murow[:, :qw], in0=psr[0:1, :qw], scalar=1.0 / D,
                in1=muxr[0:1, gsl], op0=mybir.AluOpType.mult, op1=ADD)
            mu_bf = rows.tile([1, 512], BF16, tag="rowb", name="mu2bf")
            with ALP(reason="bf16 rows"):
                nc.vector.tensor_copy(out=mu_bf[:, :qw], in_=murow[:, :qw])
            mu2_bc = bcp.tile([P, 512], F32, tag="bc", name="mu2_bc")
            psb1 = ps_ref[0].tile([P, 512], F32, tag="mm", name="psb_mu2")
            nc.tensor.matmul(psb1[:, :qw], onesb_r, mu_bf[:, :qw],
                             start=True, stop=True)
            nc.vector.tensor_copy(out=mu2_bc[:, :qw], in_=psb1[:, :qw])
            return murow, mu2_bc

        def emit_ln2_s2(ic, q0, qw, murow, mu2_bc):
            sq = slice(q0, q0 + qw)
            # mu^2 on ACT (idle in the tail window); var fused via stt
            musq = rows.tile([1, 512], F32, tag="row", name="musq")
            nc.scalar.activation(out=musq[:, :qw], in_=murow[:, :qw],
                                 func=AF.Square, scale=1.0)
            pss = ps_ref[0].tile([32, 512], F32, tag="mm", name="ps_s22")
            dr_group(pss[:, :qw], lambda kp: ones8,
                     lambda kp: x2s[:, 2 * kp:2 * kp + 2, sq], KP)
            var = rows.tile([1, 512], F32, tag="row", name="var2")
            nc.vector.scalar_tensor_tensor(
                out=var[:, :qw], in0=pss[0:1, :qw], scalar=1.0 / D,
                in1=musq[:, :qw], op0=mybir.AluOpType.mult,
                op1=mybir.AluOpType.subtract)
            rstd2 = rows.tile([1, 512], F32, tag="row", name="rstd2")
            nc.scalar.activation(out=rstd2[:, :qw], in_=var[:, :qw],
                                 func=AF.Sqrt, bias=eps_sb[0:1, :], scale=1.0)
            nc.vector.reciprocal(out=rstd2[:, :qw], in_=rstd2[:, :qw])
            rstd2_bc = bcp.tile([P, 512], F32, tag="bc", name="rstd2_bc")
            psb2 = ps_ref[0].tile([P, 512], F32, tag="mm", name="psb_rs2")
            nc.tensor.matmul(psb2[:, :qw], onesb_rf, rstd2[:, :qw],
                             start=True, stop=True)
            nc.vector.tensor_copy(out=rstd2_bc[:, :qw], in_=psb2[:, :qw])
            return mu2_bc, rstd2_bc

        def emit_ln2_stats(ic, q0, qw):
            murow, mu2_bc = emit_ln2_mu(ic, q0, qw)
            return emit_ln2_s2(ic, q0, qw, murow, mu2_bc)

        h2 = [None, None]
        h2_8 = [None, None]

        def emit_ln2_apply(ic, bcs, q0, qw, dcs):
            mu2_bc, rstd2_bc = bcs
            sl = slice(ic * 512 + q0, ic * 512 + q0 + qw)
            sq = slice(q0, q0 + qw)
            if h2[ic] is None:
                h2[ic] = mem.tile([P, DC, 512], BF16, tag="ha", name=f"h2_{ic}")
                h2_8[ic] = mem.tile([P, DC, 512], FP8, tag="h28",
                                    name=f"h28_{ic}")
            for dc in dcs:
                engA, engB = ((nc.vector, nc.gpsimd) if dc % 2 == 0
                              else (nc.gpsimd, nc.vector))
                lt = ltp.tile([P, 512], BF16, tag="lt", name="ln2_tmp")
                with ALP(reason="ln2"):
                    engA.tensor_sub(lt[:, :qw], x2_fm[:, dc, sl],
                                    mu2_bc[:, q0:q0 + qw])
                    engA.tensor_mul(h2[ic][:, dc, sq], lt[:, :qw],
                                    rstd2_bc[:, q0:q0 + qw])
                    engB.tensor_mul(h2_8[ic][:, dc, sq], lt[:, :qw],
                                    rstd2_bc[:, q0:q0 + qw])

        def emit_fc1_chunk(ic, mo, q0=0, qw=512, gdst=None):
            sq = slice(q0, q0 + qw)
            ps = ps_ref[0].tile([P, 512], F32, tag="mm", name="ps_fc1")
            if mo < HC // 2:
                dr_group(ps[:, :qw],
                         lambda kp: fc1w8_sb[:, 2 * kp:2 * kp + 2,
                                             mo * P:(mo + 1) * P],
                         lambda kp: h2_8[ic][:, 2 * kp:2 * kp + 2, sq], KP)
                scale = 1.0 / 16.0
            else:
                mb = mo - HC // 2
                for k in range(DC):
                    nc.tensor.matmul(ps[:, :qw],
                                     fc1w_sb[:, k, mb * P:(mb + 1) * P],
                                     h2[ic][:, k, sq],
                                     start=(k == 0), stop=(k == DC - 1))
                scale = 1.0
            if gdst is None:
                gdst = gelu_t[:, mo, sq]
            with ALP(reason="fp8 gelu"):
                nc.scalar.activation(out=gdst, in_=ps[:, :qw],
                                     func=AF.Gelu,
                                     bias=fc1b_sb[:, mo:mo + 1], scale=scale)

        def emit_fc2_chunk(ic, mo, q0, qw, gsrc=None):
            sl = slice(ic * 512 + q0, ic * 512 + q0 + qw)
            sq = slice(q0, q0 + qw)
            if gsrc is None:
                gsrc = lambda kp: gelu_t[:, 2 * kp:2 * kp + 2, sq]
            ps = ps_ref[0].tile([P, 512], F32, tag="mm", name="ps_fc2")
            dr_group(ps[:, :qw],
                     lambda kp: fc2w_sb[:, 2 * kp:2 * kp + 2,
                                        mo * P:(mo + 1) * P],
                     gsrc, HC // 2)
            ft = ftp.tile([P, 512], BF16, tag="ft", name="fc2_tmp")
            with ALP(reason="bf16 fc2 tmp"):
                nc.vector.tensor_scalar(out=ft[:, :qw], in0=ps[:, :qw],
                                        scalar1=1.0 / 16.0,
                                        scalar2=fc2b_sb[:, mo:mo + 1],
                                        op0=mybir.AluOpType.mult, op1=ADD)
            nc.gpsimd.tensor_add(out_fm[:, mo, sl], ft[:, :qw],
                                 x2_fm[:, mo, sl])

        def emit_exit_tr(t, tail=False):
            y_stage = mem2.tile([P, D], F32, tag="ys", name="y_stage")
            for dg in range(2):
                pt = ps_ref[0].tile([P, 3, P], F32, tag="mm", name="ps_tr2")
                for q in range(3):
                    dc = dg * 3 + q
                    nc.tensor.transpose(pt[:, q, :],
                                        out_fm[:, dc, t * P:(t + 1) * P],
                                        ident)
                eng = nc.scalar.copy if tail and dg % 2 \
                    else nc.vector.tensor_copy
                eng(out=y_stage[:, dg * 3 * P:(dg + 1) * 3 * P],
                    in_=pt.rearrange("p a b -> p (a b)"))
            nc.sync.dma_start(out=y_d[t * P:(t + 1) * P, :], in_=y_stage)

        # ---- attention + work-queue schedule ----
        wq = collections.deque()

        def drain(n):
            for _ in range(min(n, len(wq))):
                wq.popleft()()

        def refill(ic, h):
            if ic == 0:
                if h == 0:
                    for t in range(NT):
                        for vc in range(2):
                            wq.append(lambda t=t, vc=vc: emit_v_chunk(t, vc))
                elif h == 1:
                    for j in (2, 3, 8, 9):
                        for i2 in range(IC):
                            wq.append(lambda j=j, i2=i2: emit_qk_chunk(j, i2))
                elif h == 2:
                    for j in (4, 5, 10, 11):
                        for i2 in range(IC):
                            wq.append(lambda j=j, i2=i2: emit_qk_chunk(j, i2))
                if h == 7:
                    nc.sync.dma_start(out=fc1w8_sb, in_=fc1w8_v)
                elif h in (8, 9):
                    ko = 3 * (h - 8)
                    nc.sync.dma_start(out=fc1w_sb[:, ko:ko + 3, :],
                                      in_=fc1w_v[:, ko:ko + 3, :])
                elif h == 11:
                    nc.sync.dma_start(out=fc2w_sb[:, 0:4, :],
                                      in_=fc2w_v[:, 0:4, :])
            else:
                if h < 5:
                    ko = 4 * h + 4
                    nc.sync.dma_start(out=fc2w_sb[:, ko:ko + 4, :],
                                      in_=fc2w_v[:, ko:ko + 4, :])
                if h == 1:
                    # attn_fm(ic0) complete once attnV(h11, ic0) drained (h0)
                    for mo in range(DC):
                        wq.append(lambda mo=mo: emit_proj_chunk(0, mo, 0, 512))
                elif h == 2:
                    def stats0():
                        _st["bcs0"] = emit_ln2_stats(0, 0, 512)
                    wq.append(stats0)
                    wq.append(lambda: emit_ln2_apply(0, _st["bcs0"], 0, 512,
                                                     range(3)))
                    wq.append(lambda: emit_ln2_apply(0, _st["bcs0"], 0, 512,
                                                     range(3, DC)))
                elif h in (3, 4, 5, 6, 7, 8):
                    hi = 12 + 2 * (h - 3)
                    for mo in range(hi, min(hi + 2, 22)):
                        for q in range(2):
                            wq.append(lambda mo=mo, q=q:
                                      emit_fc1_chunk(0, mo, 256 * q, 256))

        def emit_attnv(h, ic, expT):
            pso = ps_att.tile([VA, 512], F32, tag="att", name="pso")
            for c in range(NT // 2):
                nc.tensor.matmul(pso, v_aug[:, 2 * c:2 * c + 2, h, :],
                                 expT[:, 2 * c:2 * c + 2, :],
                                 start=(c == 0), stop=(c == NT // 2 - 1),
                                 perf_mode=DR)
            rec = recp.tile([1, 512], BF16, tag="rec", name="rec")
            with ALP(reason="bf16 recip"):
                nc.vector.reciprocal(out=rec, in_=pso[DH:DH + 1, :])
            # PE broadcast of 1/denom into PSUM, DVE copy to SBUF
            psb = ps_ref[0].tile([P, 512], F32, tag="mm", name="psb")
            nc.tensor.matmul(psb[0:DH, :], onesb_r[:, 0:DH], rec,
                             start=True, stop=True)
            rb = rbp.tile([DH, 512], BF16, tag="rb", name="rb")
            with ALP(reason="bf16 rb"):
                nc.vector.tensor_copy(out=rb, in_=psb[0:DH, :])
            with ALP(reason="fp8 attn"):
                nc.vector.tensor_mul(
                    out=attn_fm[64 * (h % 2):64 * (h % 2) + 64,
                                h // 2, ic * 512:(ic + 1) * 512],
                    in0=pso[0:DH, :], in1=rb)

        _st = {}
        DEPTH = 1  # attnV(h) emitted after scores(h+DEPTH)
        with tc.tile_pool(name="ps_sc", bufs=2, space="PSUM") as ps_sc, \
             tc.tile_pool(name="ps_att", bufs=2, space="PSUM") as ps_att, \
             tc.tile_pool(name="expp", bufs=DEPTH + 1) as expp:
            for j in (6, 7, 0, 1):
                for i2 in range(IC):
                    emit_qk_chunk(j, i2, act=(j >= 6))
            pend = collections.deque()
            for ic in range(IC):
                for h in range(H):
                    refill(ic, h)
                    g, b = divmod(h, 4)
                    p0 = 32 * b
                    expT = expp.tile([P, NT, 512], FP8, tag="ex", name="expT")
                    for jp in range(NT // 2):
                        ps = ps_sc.tile([P, 2, 512], F32, tag="sc",
                                        name="ps_sc")
                        for half in range(2):
                            jc = 2 * jp + half
                            nc.tensor.matmul(
                                ps[:, half, :],
                                k_fold[p0:p0 + 32, g, :, jc * P:(jc + 1) * P],
                                q_fold[p0:p0 + 32, g, :,
                                       ic * 512:(ic + 1) * 512],
                                start=True, stop=True, perf_mode=DR,
                                tile_position=(p0, 0))
                        with ALP(reason="fp8 exp"):
                            nc.scalar.activation(
                                out=expT[:, 2 * jp:2 * jp + 2, :], in_=ps,
                                func=AF.Exp, scale=0.125)
                        drain(2 if h < 5 else 1)
                    pend.append((h, ic, expT))
                    if len(pend) > DEPTH:
                        emit_attnv(*pend.popleft())
                    drain(2)
            dq = collections.deque(
                list(range(12)) + [(22, 0), (22, 1), (23, 0), (23, 1)])
            while pend:
                emit_attnv(*pend.popleft())
                drain(2)
                for _ in range(4):
                    if dq:
                        it = dq.popleft()
                        if isinstance(it, tuple):
                            emit_fc1_chunk(0, it[0], 256 * it[1], 256)
                        else:
                            emit_fc1_chunk(0, it)
            drain(len(wq))
            while dq:
                it = dq.popleft()
                if isinstance(it, tuple):
                    emit_fc1_chunk(0, it[0], 256 * it[1], 256)
                else:
                    emit_fc1_chunk(0, it)

        # attention PSUM pools closed: open a wide tail pool (6 banks)
        with tc.tile_pool(name="ps_tail", bufs=6, space="PSUM") as ps_tail:
            ps_ref[0] = ps_tail
            # ---- tail: ic1 MLP ----
            mu1 = emit_ln2_mu(1, 0, 512)
            for mo in range(DC):
                emit_proj_chunk(1, mo, 0, 512)
            bcs = emit_ln2_s2(1, 0, 256, *mu1)
            bcs_b1 = emit_ln2_s2(1, 0, 512, *mu1)
            emit_ln2_apply(1, bcs, 0, 256, range(DC))
            # band0 gelus go to a buffer aliased on dead v_aug memory, so
            # fc1(ic1) needn't wait for fc2(ic0) to drain gelu_t
            gelu_b0 = mem.tile([P, HC, 256], FP8, tag="vf", name="gelu_b0")
            for mo in range(HC):
                emit_fc1_chunk(1, mo, 0, 256, gdst=gelu_b0[:, mo, :])
                if mo % 2 == 1 and mo < 12:
                    emit_fc2_chunk(0, mo // 2, 0, 512)
                elif mo >= 12 and mo % 3 == 2:
                    emit_exit_tr((mo - 12) // 3)
                if mo == 11:
                    emit_ln2_apply(1, bcs_b1, 256, 256, range(DC))
            for mo in range(HC):
                emit_fc1_chunk(1, mo, 256, 256,
                               gdst=gelu_t[:, mo, 0:256])
                if mo % 4 == 3:
                    emit_fc2_chunk(
                        1, mo // 4, 0, 256,
                        gsrc=lambda kp: gelu_b0[:, 2 * kp:2 * kp + 2, :])
            for t in (4, 5):
                emit_exit_tr(t, tail=True)
            for mo in range(DC):
                emit_fc2_chunk(
                    1, mo, 256, 256,
                    gsrc=lambda kp: gelu_t[:, 2 * kp:2 * kp + 2, 0:256])
            for t in (6, 7):
                emit_exit_tr(t, tail=True)



_NC_CACHE = {}


def _get_nc(qkb_zero=False):
    key = ("nc", qkb_zero)
    if key not in _NC_CACHE:
        _NC_CACHE[key] = build_nc(qkb_zero)
    return _NC_CACHE[key]


def _fold_perm():
    perm = []
    for g in range(3):
        for half in range(2):
            for hh in range(4):
                h = 4 * g + hh
                perm.extend(range(h * 64 + 32 * half, h * 64 + 32 * half + 32))
    return np.asarray(perm)


def _prep_inputs(x, ln1_g, ln1_b, qkv_w, qkv_b, proj_w, proj_b,
                 ln2_g, ln2_b, fc1_w, fc1_b, fc2_w, fc2_b):
    f = lambda a: np.asarray(a, np.float32)
    x = f(x)
    qkv_w, qkv_b = f(qkv_w), f(qkv_b)
    proj_w, proj_b = f(proj_w), f(proj_b)
    fc1_w, fc1_b = f(fc1_w), f(fc1_b)
    fc2_w, fc2_b = f(fc2_w), f(fc2_b)
    ln1_g, ln1_b, ln2_g, ln2_b = f(ln1_g), f(ln1_b), f(ln2_g), f(ln2_b)

    # fold LN affine into the following matmul
    qkv_w_eff = ln1_g[:, None] * qkv_w
    qkv_b_eff = qkv_b + ln1_b @ qkv_w
    fc1_w_eff = ln2_g[:, None] * fc1_w
    fc1_b_eff = (fc1_b + ln2_b @ fc1_w).astype(np.float32)

    # v bias commutes through softmax -> fold into proj bias
    vb = qkv_b_eff[2 * D:]
    proj_b_eff = (proj_b + vb @ proj_w).astype(np.float32)

    # fold permutation for q/k DoubleRow scores
    perm = _fold_perm()
    wq = qkv_w_eff[:, 0:D][:, perm]
    wk = qkv_w_eff[:, D:2 * D][:, perm]
    wv = qkv_w_eff[:, 2 * D:]
    wqkv = np.concatenate([wq, wk, wv], axis=1).astype(FP8_NP)
    qkb = np.concatenate([qkv_b_eff[0:D][perm],
                          qkv_b_eff[D:2 * D][perm]]).astype(np.float32)

    proj_w8 = proj_w.astype(FP8_NP)
    wsum8 = np.repeat(proj_w8.astype(np.float32).sum(axis=1, keepdims=True),
                      32, axis=1).astype(FP8_NP)
    pbsum = proj_b_eff.sum() / D
    biases = np.concatenate([
        qkb, proj_b_eff, fc1_b_eff, fc2_b.astype(np.float32),
        np.full(P, pbsum, np.float32)]).astype(np.float32)
    shared = {
        "wqkv": wqkv, "wsum8": wsum8, "biases": biases,
        "proj_w": proj_w8,
        "fc1_w": fc1_w_eff[:, HID // 2:].astype(BF16_NP),
        "fc1_w8": (16.0 * fc1_w_eff[:, :HID // 2]).astype(FP8_NP),
        "fc2_w": (16.0 * fc2_w).astype(FP8_NP),
    }
    n_cores = x.shape[0]
    return [{"x": np.ascontiguousarray(x[c]).astype(BF16_NP), **shared}
            for c in range(n_cores)]


def kernel(**inputs):
    in_maps = _prep_inputs(**inputs)
    nc = _get_nc(
        qkb_zero=bool(np.all(in_maps[0]["biases"][:2 * D] == 0.0)))
    res = run_bass_kernel_spmd(nc, in_maps, core_ids=list(range(len(in_maps))))
    return np.stack([r["y"] for r in res.results], axis=0)


if __name__ == "__main__":
    import reference
    inputs = {k: np.asarray(v) for k, v in reference.setup_inputs().items()}
    out = kernel(**inputs)
    print("kernel out", out.shape, out.dtype)



# revision 112
# speedup vs baseline: 1.0289x; 1.0024x over previous
"""Trainium2 Bass kernel for a dense transformer block (PreNorm attn + MLP).

Full inputs: x [8, 1024, 768] f32 + LN/attn/MLP weights.
Sharding: pure data-parallel — batch 8 across 8 NeuronCores, no collectives.

Per-core design (tokens n=1024, d=768, heads=12, dh=64, hidden=3072):
  - Residual spine fp32 (x2) / bf16 (x), FEATURE-major; weights [d_in, d_out]
    serve as lhsT directly.
  - Deep matmuls fp8e4 DoubleRow (0.5 cyc/row contracting two 128-k-tiles);
    fc1 is MIXED: hidden 0:1536 fp8 DR (weights x16, gelu scale 1/16),
    1536:3072 bf16 — full-fp8 fc1 would blow the 2e-2 error budget.
  - Scores: q/k "folded" — head h on 32 partitions, dh split into the 2
    DoubleRow k-tiles (host-side qkv_w column permutation makes this free).
  - attnV: lhsT = v_aug [j, 2, 96]: 64 v dims + ones col (softmax
    denominator) + 31 zero pad; v bias folded into proj_b host-side.
    1/denom broadcast via PE ones-matmul + DVE copy (gpsimd can't touch
    PSUM; 0-stride-partition SBUF DMA is rejected).
  - LN1 is TOKEN-major: bn_stats/bn_aggr per 128-token block as x lands,
    h1 = tensor_scalar((x-mu)*rstd) per block, then PE-transposed to
    feature-major (x first — it only needs the DMA; h1 second).  Kills the
    old stats->broadcast->apply chain.  pbsum is folded into muxr.
  - exp on ACT is THE bottleneck (~100us); scheduling keeps ACT hot:
    ic0 MLP (proj/LN2/fc1) drains between score-pairs of ic1's exp stream
    (bf16 fc1 split 256-wide to fit the 1.04us exp cadence); the 12 fp8
    fc1(ic0) chunks + 4 bf16 quarters are emitted in the attnV flush so
    their gelus seamlessly extend the exp stream.
  - Tail: attention PSUM pools close, a 6-buf ps_tail pool opens (psum
    rotation was the pacing limit); LN2 mu from the wsum trick before proj,
    s2/var fused via ACT Square + stt, f32 PE broadcast; band0 (256 tok)
    stats fast-path + full-width pass for band1; band0 gelus into a buffer
    aliased on dead v_aug so fc1(ic1) needn't wait for fc2(ic0); band1 fc1
    interleaves with fc2(ic1) tq0 to keep the gelu stream continuous.
  - DMAs coalesced (26 total): x in 4, each weight matrix 1-2, biases
    packed into one [128,49] tensor host-side.
"""

import collections

import numpy as np
import ml_dtypes

import concourse.bass as bass
import concourse.tile as tile
from concourse import mybir
from concourse.masks import make_identity
from concourse.bass_utils import run_bass_kernel_spmd

F32 = mybir.dt.float32
BF16 = mybir.dt.bfloat16
FP8 = mybir.dt.float8e4
BF16_NP = ml_dtypes.bfloat16
FP8_NP = ml_dtypes.float8_e4m3
AF = mybir.ActivationFunctionType
DR = mybir.MatmulPerfMode.DoubleRow
ADD = mybir.AluOpType.add

N = 1024          # tokens per core
D = 768           # model dim
H = 12            # heads
DH = 64           # head dim
HID = 3072        # mlp hidden
P = 128
NT = N // P       # 8 token chunks
DC = D // P       # 6 feature chunks
HC = HID // P     # 24 hidden chunks
KP = DC // 2      # 3 contraction pairs for d=768
IC = 2            # token halves of 512
VA = 96           # attnV stationary cols: 64 v + ones + 31 pad
EPS = 1e-5


def build_nc(qkb_zero=False):
    nc = bass.Bass("TRN2")

    x_d = nc.dram_tensor("x", [N, D], BF16, kind="ExternalInput")
    wqkv_d = nc.dram_tensor("wqkv", [D, 3 * D], FP8, kind="ExternalInput")
    projw_d = nc.dram_tensor("proj_w", [D, D], FP8, kind="ExternalInput")
    wsum_d = nc.dram_tensor("wsum8", [D, 32], FP8, kind="ExternalInput")
    biases_d = nc.dram_tensor("biases", [49 * P], F32, kind="ExternalInput")
    fc1w_d = nc.dram_tensor("fc1_w", [D, HID // 2], BF16,
                            kind="ExternalInput")
    fc1w8_d = nc.dram_tensor("fc1_w8", [D, HID // 2], FP8,
                             kind="ExternalInput")
    fc2w_d = nc.dram_tensor("fc2_w", [HID, D], FP8, kind="ExternalInput")
    y_d = nc.dram_tensor("y", [N, D], F32, kind="ExternalOutput")

    with tile.TileContext(nc) as tc:
        _body(nc, tc, x_d, wqkv_d, biases_d, projw_d,
              fc1w_d, fc1w8_d, fc2w_d, y_d, qkb_zero, wsum_d)
    # this container's walrus accepts at most 1 sync wait per instruction
    # (2 on EventSemaphore); redistribute excess waits like Bacc.compile does
    import bass_rust as _br
    _br.move_matmul_waits_to_ldweights(nc.m)
    _br.generate_event_semaphores(nc)
    return nc


def _body(nc, tc, x_d, wqkv_d, biases_d, projw_d,
          fc1w_d, fc1w8_d, fc2w_d, y_d, qkb_zero, wsum_d):
    from contextlib import ExitStack
    with ExitStack() as ctx:
        consts = ctx.enter_context(tc.tile_pool(name="consts", bufs=1))
        rows = ctx.enter_context(tc.tile_pool(name="rows", bufs=3))
        recp = ctx.enter_context(tc.tile_pool(name="recp", bufs=3))
        rbp = ctx.enter_context(tc.tile_pool(name="rbp", bufs=3))
        ltp = ctx.enter_context(tc.tile_pool(name="ltp", bufs=4))
        ftp = ctx.enter_context(tc.tile_pool(name="ftp", bufs=3))
        bcp = ctx.enter_context(tc.tile_pool(name="bcp", bufs=3))
        dram = ctx.enter_context(tc.tile_pool(name="dram", bufs=1, space="DRAM"))
        ps_mm = ctx.enter_context(tc.tile_pool(name="ps_mm", bufs=2, space="PSUM"))
        ps_ref = [ps_mm]
        mem = ctx.enter_context(tc.tile_pool(name="mem", bufs=1))
        mem2 = ctx.enter_context(tc.tile_pool(name="mem2", bufs=2))

        ALP = nc.allow_low_precision

        # ---- constants & early DMAs (x first, then q/k weights) ----
        ident = consts.tile([P, P], F32, tag="ident")
        make_identity(nc, ident)
        ident_bf = consts.tile([P, P], BF16, tag="ident_bf")
        make_identity(nc, ident_bf)

        x_tok = mem.tile([P, NT, D], BF16, tag="xo")           # -> out_fm later
        x_view = x_d[:, :].rearrange("(t p) d -> p t d", p=P)
        for tp in range(4):
            nc.sync.dma_start(out=x_tok[:, 2 * tp:2 * tp + 2, :],
                              in_=x_view[:, 2 * tp:2 * tp + 2, :])

        wqkv_sb = mem.tile([P, DC, 3 * D], FP8, tag="w1")      # -> fc1w later
        wqkv_v = wqkv_d[:, :].rearrange("(ko p) m -> p ko m", p=P)
        nc.sync.dma_start(out=wqkv_sb[:, :, 0:2 * D],
                          in_=wqkv_v[:, :, 0:2 * D])

        ones8 = consts.tile([P, 2, 32], FP8, tag="ones8")
        nc.vector.memset(ones8, 1.0)
        onesb_c = consts.tile([P, 1], BF16, tag="onesb_c")     # stats lhsT
        nc.vector.memset(onesb_c, 1.0)
        onesb_r = consts.tile([1, P], BF16, tag="onesb_r")     # bcast lhsT
        nc.vector.memset(onesb_r, 1.0)
        onesb_rf = consts.tile([1, P], F32, tag="onesb_rf")    # f32 bcast
        nc.vector.memset(onesb_rf, 1.0)
        eps_sb = consts.tile([P, 1], F32, tag="eps_sb")
        nc.vector.memset(eps_sb, EPS)

        # biases packed host-side: cols 0:12 qkb, 12:18 projb, 18:42 fc1b,
        # 42:48 fc2b, 48 pbsum (broadcast)
        bias_all = consts.tile([P, 49], F32, tag="biases")
        nc.sync.dma_start(out=bias_all,
                          in_=biases_d[:].rearrange("(mo p) -> p mo", p=P))
        qkb_sb = bias_all[:, 0:12]
        projb_sb = bias_all[:, 12:18]
        fc1b_sb = bias_all[:, 18:42]
        fc2b_sb = bias_all[:, 42:48]
        pbs_sb = bias_all[0:1, 48:49]

        # deferred weight DMAs (after x / qk in the DMA queues)
        nc.sync.dma_start(out=wqkv_sb[:, :, 2 * D:],
                          in_=wqkv_v[:, :, 2 * D:])
        wsum_sb = consts.tile([P, DC, 32], FP8, tag="wsum")
        nc.sync.dma_start(out=wsum_sb,
                          in_=wsum_d[:, :].rearrange("(ko p) m -> p ko m", p=P))
        projw_sb = mem.tile([P, DC, D], FP8, tag="pw")
        projw_v = projw_d[:, :].rearrange("(ko p) m -> p ko m", p=P)
        nc.sync.dma_start(out=projw_sb, in_=projw_v)

        # v_aug: ones col + zero pad (finite garbage would still poison psum)
        v_aug = mem.tile([P, NT, H, VA], FP8, tag="vf")

        # ---- lead-in: token-major LN1 (per-token stats on free axis) ----
        x_fm = mem.tile([P, DC, N], BF16, tag="xf")
        muxr = mem.tile([1, N], BF16, tag="mux")
        h1 = mem.tile([P, DC, N], FP8, tag="ha")               # -> h2 later
        h1_tok = mem.tile([P, NT, D], BF16, tag="ge")      # -> gelu_t later
        mv_tok = mem.tile([P, NT, 2], F32, tag="mvt")
        rstd_tok = mem.tile([P, NT], F32, tag="rst")

        SUB = mybir.AluOpType.subtract
        MUL = mybir.AluOpType.mult
        for t in range(NT):
            bs = rows.tile([P, 2, 6], F32, tag="bs", name="bn")
            nc.vector.bn_stats(bs[:, 0, :], x_tok[:, t, 0:384])
            nc.vector.bn_stats(bs[:, 1, :], x_tok[:, t, 384:768])
            nc.vector.bn_aggr(mv_tok[:, t, :], bs)
            if t % 4 == 3:
                hf = t // 4
                nc.scalar.activation(
                    out=rstd_tok[:, 4 * hf:4 * hf + 4],
                    in_=mv_tok[:, 4 * hf:4 * hf + 4, 1],
                    func=AF.Sqrt, bias=eps_sb, scale=1.0)
                nc.vector.reciprocal(
                    out=rstd_tok[:, 4 * hf:4 * hf + 4],
                    in_=rstd_tok[:, 4 * hf:4 * hf + 4])
                for tt in range(4 * hf, 4 * hf + 4):
                    with ALP(reason="fp8 h1"):
                        nc.vector.tensor_scalar(
                            out=h1_tok[:, tt, :], in0=x_tok[:, tt, :],
                            scalar1=mv_tok[:, tt, 0:1],
                            scalar2=rstd_tok[:, tt:tt + 1],
                            op0=SUB, op1=MUL)

        nc.gpsimd.memset(v_aug[:, :, :, DH + 1:], 0.0)
        nc.gpsimd.memset(v_aug[:, :, :, DH:DH + 1], 1.0)
        # transposes to feature-major: x first (only needs the DMA), h1
        # second (gated by the per-token stats chain)
        with tc.tile_pool(name="ps_lead", bufs=6, space="PSUM") as ps_lead:
            for hf in range(2):
                sl = slice(hf * 512, (hf + 1) * 512)
                for dc in range(DC):
                    pt = ps_lead.tile([P, 4, P], BF16, tag="tr", name="ptx")
                    for q in range(4):
                        t = hf * 4 + q
                        nc.tensor.transpose(
                            pt[:, q, :], x_tok[:, t, dc * P:(dc + 1) * P],
                            ident_bf)
                    with ALP(reason="bf16 x_fm"):
                        nc.scalar.copy(
                            out=x_fm[:, dc, sl],
                            in_=pt.rearrange("p a b -> p (a b)"))
            for hf in range(2):
                sl = slice(hf * 512, (hf + 1) * 512)
                for dc in range(DC):
                    ph = ps_lead.tile([P, 4, P], BF16, tag="tr", name="pth")
                    for q in range(4):
                        t = hf * 4 + q
                        nc.tensor.transpose(
                            ph[:, q, :], h1_tok[:, t, dc * P:(dc + 1) * P],
                            ident_bf)
                    heng = nc.scalar.copy if dc % 2 else nc.vector.tensor_copy
                    with ALP(reason="fp8 h1 fm"):
                        heng(out=h1[:, dc, sl],
                             in_=ph.rearrange("p a b -> p (a b)"))
                    if dc == DC - 1:
                        psm = ps_lead.tile([1, 512], F32, tag="tr",
                                           name="ps_mu1")
                        for k in range(DC):
                            nc.tensor.matmul(psm, onesb_c, x_fm[:, k, sl],
                                             start=(k == 0),
                                             stop=(k == DC - 1))
                        with ALP(reason="bf16 mux"):
                            nc.vector.tensor_scalar(
                                out=muxr[0:1, sl], in0=psm, scalar1=1.0 / D,
                                scalar2=pbs_sb,
                                op0=mybir.AluOpType.mult, op1=ADD)


        # ---- DoubleRow helpers ----
        def dr_group(ps_ap, lhs_fn, rhs_fn, nkp):
            for kp in range(nkp):
                nc.tensor.matmul(ps_ap, lhs_fn(kp), rhs_fn(kp),
                                 start=(kp == 0), stop=(kp == nkp - 1),
                                 perf_mode=DR)

        q_fold = mem.tile([P, 3, 2, N], FP8, tag="qf")
        k_fold = mem.tile([P, 3, 2, N], FP8, tag="kf")

        def emit_qk_chunk(j, ic, act=False):
            """j in 0..11: q chunks 0-5 as (g, half), k chunks 6-11."""
            g, half = divmod(j % 6, 2)
            dst = k_fold if j >= 6 else q_fold
            sl = slice(ic * 512, (ic + 1) * 512)
            ps = ps_ref[0].tile([P, 512], F32, tag="mm", name="ps_qk")
            dr_group(ps,
                     lambda kp: wqkv_sb[:, 2 * kp:2 * kp + 2, j * P:(j + 1) * P],
                     lambda kp: h1[:, 2 * kp:2 * kp + 2, sl], KP)
            with ALP(reason="fp8 qk"):
                if act and qkb_zero:
                    nc.scalar.copy(out=dst[:, g, half, sl], in_=ps)
                elif qkb_zero:
                    nc.vector.tensor_copy(out=dst[:, g, half, sl], in_=ps)
                else:
                    nc.vector.tensor_scalar_add(out=dst[:, g, half, sl],
                                                in0=ps,
                                                scalar1=qkb_sb[:, j:j + 1])

        def emit_v_chunk(t, vc):
            fw = 512 if vc == 0 else 256
            ps = ps_ref[0].tile([P, 512], F32, tag="mm", name="ps_v")
            dr_group(ps[:, :fw],
                     lambda kp: h1[:, 2 * kp:2 * kp + 2, t * P:(t + 1) * P],
                     lambda kp: wqkv_sb[:, 2 * kp:2 * kp + 2,
                                        2 * D + vc * 512:2 * D + vc * 512 + fw],
                     KP)
            with ALP(reason="fp8 v"):
                nc.vector.tensor_copy(
                    out=v_aug[:, t, vc * 8:vc * 8 + fw // DH, 0:DH],
                    in_=ps[:, :fw].rearrange("p (h e) -> p h e", e=DH))

        fc1w_sb = mem.tile([P, DC, HID // 2], BF16, tag="w1")
        fc1w_v = fc1w_d[:, :].rearrange("(ko p) m -> p ko m", p=P)
        fc1w8_sb = mem.tile([P, DC, HID // 2], FP8, tag="w18")
        fc1w8_v = fc1w8_d[:, :].rearrange("(ko p) m -> p ko m", p=P)
        fc2w_sb = mem.tile([P, HC, D], FP8, tag="f2")
        fc2w_v = fc2w_d[:, :].rearrange("(ko p) m -> p ko m", p=P)

        x2_fm = mem.tile([P, DC, N], F32, tag="x2")
        attn_fm = mem.tile([P, DC, N], FP8, tag="at")
        x2s = mem.tile([P, DC, 512], FP8, tag="xq")
        gelu_t = mem.tile([P, HC, 512], FP8, tag="ge")
        out_fm = mem.tile([P, DC, N], F32, tag="xo")

        def emit_proj_chunk(ic, mo, q0, qw):
            """token window [ic*512+q0, +qw); x2c/x2s live at [q0, q0+qw)."""
            sl = slice(ic * 512 + q0, ic * 512 + q0 + qw)
            sq = slice(q0, q0 + qw)
            ps = ps_ref[0].tile([P, 512], F32, tag="mm", name="ps_proj")
            dr_group(ps[:, :qw],
                     lambda kp: projw_sb[:, 2 * kp:2 * kp + 2,
                                         mo * P:(mo + 1) * P],
                     lambda kp: attn_fm[:, 2 * kp:2 * kp + 2, sl], KP)
            nc.vector.scalar_tensor_tensor(
                out=x2_fm[:, mo, sl], in0=ps[:, :qw],
                scalar=projb_sb[:, mo:mo + 1], in1=x_fm[:, mo, sl],
                op0=ADD, op1=ADD)
            with ALP(reason="fp8 stats"):
                enq = nc.gpsimd if mo % 2 else nc.vector
                enq.tensor_mul(x2s[:, mo, sq], x2_fm[:, mo, sl],
                               x2_fm[:, mo, sl])

        def emit_ln2_mu(ic, q0, qw):
            gsl = slice(ic * 512 + q0, ic * 512 + q0 + qw)
            murow = rows.tile([1, 512], F32, tag="row", name="mu2row")
            psr = ps_ref[0].tile([32, 512], F32, tag="mm", name="ps_mu2")
            dr_group(psr[:, :qw], lambda kp: wsum_sb[:, 2 * kp:2 * kp + 2, :],
                     lambda kp: attn_fm[:, 2 * kp:2 * kp + 2, gsl], KP)
            # muxr carries mean_d(x) + pbsum (folded at lead-in)
            nc.vector.scalar_tensor_tensor(
                out=murow[:, :qw], in0=psr[0:1, :qw], scalar=1.0 / D,
                in1=muxr[0:1, gsl], op0=mybir.AluOpType.mult, op1=ADD)
            mu_bf = rows.tile([1, 512], BF16, tag="rowb", name="mu2bf")
            with ALP(reason="bf16 rows"):
                nc.vector.tensor_copy(out=mu_bf[:, :qw], in_=murow[:, :qw])
            mu2_bc = bcp.tile([P, 512], F32, tag="bc", name="mu2_bc")
            psb1 = ps_ref[0].tile([P, 512], F32, tag="mm", name="psb_mu2")
            nc.tensor.matmul(psb1[:, :qw], onesb_r, mu_bf[:, :qw],
                             start=True, stop=True)
            nc.vector.tensor_copy(out=mu2_bc[:, :qw], in_=psb1[:, :qw])
            return murow, mu2_bc

        def emit_ln2_s2(ic, q0, qw, murow, mu2_bc):
            sq = slice(q0, q0 + qw)
            # mu^2 on ACT (idle in the tail window); var fused via stt
            musq = rows.tile([1, 512], F32, tag="row", name="musq")
            nc.scalar.activation(out=musq[:, :qw], in_=murow[:, :qw],
                                 func=AF.Square, scale=1.0)
            pss = ps_ref[0].tile([32, 512], F32, tag="mm", name="ps_s22")
            dr_group(pss[:, :qw], lambda kp: ones8,
                     lambda kp: x2s[:, 2 * kp:2 * kp + 2, sq], KP)
            var = rows.tile([1, 512], F32, tag="row", name="var2")
            nc.vector.scalar_tensor_tensor(
                out=var[:, :qw], in0=pss[0:1, :qw], scalar=1.0 / D,
                in1=musq[:, :qw], op0=mybir.AluOpType.mult,
                op1=mybir.AluOpType.subtract)
            rstd2 = rows.tile([1, 512], F32, tag="row", name="rstd2")
            nc.scalar.activation(out=rstd2[:, :qw], in_=var[:, :qw],
                                 func=AF.Sqrt, bias=eps_sb[0:1, :], scale=1.0)
            nc.vector.reciprocal(out=rstd2[:, :qw], in_=rstd2[:, :qw])
            rstd2_bc = bcp.tile([P, 512], F32, tag="bc", name="rstd2_bc")
            psb2 = ps_ref[0].tile([P, 512], F32, tag="mm", name="psb_rs2")
            nc.tensor.matmul(psb2[:, :qw], onesb_rf, rstd2[:, :qw],
                             start=True, stop=True)
            nc.vector.tensor_copy(out=rstd2_bc[:, :qw], in_=psb2[:, :qw])
            return mu2_bc, rstd2_bc

        def emit_ln2_stats(ic, q0, qw):
            murow, mu2_bc = emit_ln2_mu(ic, q0, qw)
            return emit_ln2_s2(ic, q0, qw, murow, mu2_bc)

        h2 = [None, None]
        h2_8 = [None, None]

        def emit_ln2_apply(ic, bcs, q0, qw, dcs):
            mu2_bc, rstd2_bc = bcs
            sl = slice(ic * 512 + q0, ic * 512 + q0 + qw)
            sq = slice(q0, q0 + qw)
            if h2[ic] is None:
                h2[ic] = mem.tile([P, DC, 512], BF16, tag="ha", name=f"h2_{ic}")
                h2_8[ic] = mem.tile([P, DC, 512], FP8, tag="h28",
                                    name=f"h28_{ic}")
            for dc in dcs:
                engA, engB = ((nc.vector, nc.gpsimd) if dc % 2 == 0
                              else (nc.gpsimd, nc.vector))
                lt = ltp.tile([P, 512], BF16, tag="lt", name="ln2_tmp")
                with ALP(reason="ln2"):
                    engA.tensor_sub(lt[:, :qw], x2_fm[:, dc, sl],
                                    mu2_bc[:, q0:q0 + qw])
                    engA.tensor_mul(h2[ic][:, dc, sq], lt[:, :qw],
                                    rstd2_bc[:, q0:q0 + qw])
                    engB.tensor_mul(h2_8[ic][:, dc, sq], lt[:, :qw],
                                    rstd2_bc[:, q0:q0 + qw])

        def emit_fc1_chunk(ic, mo, q0=0, qw=512, gdst=None):
            sq = slice(q0, q0 + qw)
            ps = ps_ref[0].tile([P, 512], F32, tag="mm", name="ps_fc1")
            if mo < HC // 2:
                dr_group(ps[:, :qw],
                         lambda kp: fc1w8_sb[:, 2 * kp:2 * kp + 2,
                                             mo * P:(mo + 1) * P],
                         lambda kp: h2_8[ic][:, 2 * kp:2 * kp + 2, sq], KP)
                scale = 1.0 / 16.0
            else:
                mb = mo - HC // 2
                for k in range(DC):
                    nc.tensor.matmul(ps[:, :qw],
                                     fc1w_sb[:, k, mb * P:(mb + 1) * P],
                                     h2[ic][:, k, sq],
                                     start=(k == 0), stop=(k == DC - 1))
                scale = 1.0
            if gdst is None:
                gdst = gelu_t[:, mo, sq]
            with ALP(reason="fp8 gelu"):
                nc.scalar.activation(out=gdst, in_=ps[:, :qw],
                                     func=AF.Gelu,
                                     bias=fc1b_sb[:, mo:mo + 1], scale=scale)

        def emit_fc2_chunk(ic, mo, q0, qw, gsrc=None):
            sl = slice(ic * 512 + q0, ic * 512 + q0 + qw)
            sq = slice(q0, q0 + qw)
            if gsrc is None:
                gsrc = lambda kp: gelu_t[:, 2 * kp:2 * kp + 2, sq]
            ps = ps_ref[0].tile([P, 512], F32, tag="mm", name="ps_fc2")
            dr_group(ps[:, :qw],
                     lambda kp: fc2w_sb[:, 2 * kp:2 * kp + 2,
                                        mo * P:(mo + 1) * P],
                     gsrc, HC // 2)
            ft = ftp.tile([P, 512], BF16, tag="ft", name="fc2_tmp")
            with ALP(reason="bf16 fc2 tmp"):
                nc.vector.tensor_scalar(out=ft[:, :qw], in0=ps[:, :qw],
                                        scalar1=1.0 / 16.0,
                                        scalar2=fc2b_sb[:, mo:mo + 1],
                                        op0=mybir.AluOpType.mult, op1=ADD)
            nc.gpsimd.tensor_add(out_fm[:, mo, sl], ft[:, :qw],
                                 x2_fm[:, mo, sl])

        def emit_exit_tr(t, tail=False):
            y_stage = mem2.tile([P, D], F32, tag="ys", name="y_stage")
            for dg in range(2):
                pt = ps_ref[0].tile([P, 3, P], F32, tag="mm", name="ps_tr2")
                for q in range(3):
                    dc = dg * 3 + q
                    nc.tensor.transpose(pt[:, q, :],
                                        out_fm[:, dc, t * P:(t + 1) * P],
                                        ident)
                eng = nc.scalar.copy if tail and dg % 2 \
                    else nc.vector.tensor_copy
                eng(out=y_stage[:, dg * 3 * P:(dg + 1) * 3 * P],
                    in_=pt.rearrange("p a b -> p (a b)"))
            nc.sync.dma_start(out=y_d[t * P:(t + 1) * P, :], in_=y_stage)

        # ---- attention + work-queue schedule ----
        wq = collections.deque()

        def drain(n):
            for _ in range(min(n, len(wq))):
                wq.popleft()()

        def refill(ic, h):
            if ic == 0:
                if h == 0:
                    for t in range(NT):
                        for vc in range(2):
                            wq.append(lambda t=t, vc=vc: emit_v_chunk(t, vc))
                elif h == 1:
                    for j in (2, 3, 8, 9):
                        for i2 in range(IC):
                            wq.append(lambda j=j, i2=i2: emit_qk_chunk(j, i2))
                elif h == 2:
                    for j in (4, 5, 10, 11):
                        for i2 in range(IC):
                            wq.append(lambda j=j, i2=i2: emit_qk_chunk(j, i2))
                if h == 7:
                    nc.sync.dma_start(out=fc1w8_sb, in_=fc1w8_v)
                elif h in (8, 9):
                    ko = 3 * (h - 8)
                    nc.sync.dma_start(out=fc1w_sb[:, ko:ko + 3, :],
                                      in_=fc1w_v[:, ko:ko + 3, :])
                elif h == 11:
                    nc.sync.dma_start(out=fc2w_sb[:, 0:4, :],
                                      in_=fc2w_v[:, 0:4, :])
            else:
                if h < 5:
                    ko = 4 * h + 4
                    nc.sync.dma_start(out=fc2w_sb[:, ko:ko + 4, :],
                                      in_=fc2w_v[:, ko:ko + 4, :])
                if h == 1:
                    # attn_fm(ic0) complete once attnV(h11, ic0) drained (h0)
                    for mo in range(DC):
                        wq.append(lambda mo=mo: emit_proj_chunk(0, mo, 0, 512))
                elif h == 2:
                    def stats0():
                        _st["bcs0"] = emit_ln2_stats(0, 0, 512)
                    wq.append(stats0)
                    wq.append(lambda: emit_ln2_apply(0, _st["bcs0"], 0, 512,
                                                     range(3)))
                    wq.append(lambda: emit_ln2_apply(0, _st["bcs0"], 0, 512,
                                                     range(3, DC)))
                elif h in (3, 4, 5, 6, 7, 8):
                    hi = 12 + 2 * (h - 3)
                    for mo in range(hi, min(hi + 2, 22)):
                        for q in range(2):
                            wq.append(lambda mo=mo, q=q:
                                      emit_fc1_chunk(0, mo, 256 * q, 256))

        def emit_attnv(h, ic, expT):
            pso = ps_att.tile([VA, 512], F32, tag="att", name="pso")
            for c in range(NT // 2):
                nc.tensor.matmul(pso, v_aug[:, 2 * c:2 * c + 2, h, :],
                                 expT[:, 2 * c:2 * c + 2, :],
                                 start=(c == 0), stop=(c == NT // 2 - 1),
                                 perf_mode=DR)
            rec = recp.tile([1, 512], BF16, tag="rec", name="rec")
            with ALP(reason="bf16 recip"):
                nc.vector.reciprocal(out=rec, in_=pso[DH:DH + 1, :])
            # PE broadcast of 1/denom into PSUM, DVE copy to SBUF
            psb = ps_ref[0].tile([P, 512], F32, tag="mm", name="psb")
            nc.tensor.matmul(psb[0:DH, :], onesb_r[:, 0:DH], rec,
                             start=True, stop=True)
            rb = rbp.tile([DH, 512], BF16, tag="rb", name="rb")
            with ALP(reason="bf16 rb"):
                nc.vector.tensor_copy(out=rb, in_=psb[0:DH, :])
            with ALP(reason="fp8 attn"):
                nc.vector.tensor_mul(
                    out=attn_fm[64 * (h % 2):64 * (h % 2) + 64,
                                h // 2, ic * 512:(ic + 1) * 512],
                    in0=pso[0:DH, :], in1=rb)

        _st = {}
        DEPTH = 1  # attnV(h) emitted after scores(h+DEPTH)
        with tc.tile_pool(name="ps_sc", bufs=2, space="PSUM") as ps_sc, \
             tc.tile_pool(name="ps_att", bufs=2, space="PSUM") as ps_att, \
             tc.tile_pool(name="expp", bufs=DEPTH + 1) as expp:
            for j in (6, 7, 0, 1):
                for i2 in range(IC):
                    emit_qk_chunk(j, i2, act=(j >= 6))
            pend = collections.deque()
            for ic in range(IC):
                for h in range(H):
                    refill(ic, h)
                    g, b = divmod(h, 4)
                    p0 = 32 * b
                    expT = expp.tile([P, NT, 512], FP8, tag="ex", name="expT")
                    for jp in range(NT // 2):
                        ps = ps_sc.tile([P, 2, 512], F32, tag="sc",
                                        name="ps_sc")
                        for half in range(2):
                            jc = 2 * jp + half
                            nc.tensor.matmul(
                                ps[:, half, :],
                                k_fold[p0:p0 + 32, g, :, jc * P:(jc + 1) * P],
                                q_fold[p0:p0 + 32, g, :,
                                       ic * 512:(ic + 1) * 512],
                                start=True, stop=True, perf_mode=DR,
                                tile_position=(p0, 0))
                        with ALP(reason="fp8 exp"):
                            nc.scalar.activation(
                                out=expT[:, 2 * jp:2 * jp + 2, :], in_=ps,
                                func=AF.Exp, scale=0.125)
                        drain(2 if h < 5 else 1)
                    pend.append((h, ic, expT))
                    if len(pend) > DEPTH:
                        emit_attnv(*pend.popleft())
                    drain(2)
            dq = collections.deque(
                list(range(12)) + [(22, 0), (22, 1), (23, 0), (23, 1)])
            while pend:
                emit_attnv(*pend.popleft())
                drain(2)
                for _ in range(4):
                    if dq:
                        it = dq.popleft()
                        if isinstance(it, tuple):
                            emit_fc1_chunk(0, it[0], 256 * it[1], 256)
                        else:
                            emit_fc1_chunk(0, it)
            drain(len(wq))
            while dq:
                it = dq.popleft()
                if isinstance(it, tuple):
                    emit_fc1_chunk(0, it[0], 256 * it[1], 256)
                else:
                    emit_fc1_chunk(0, it)

        # attention PSUM pools closed: open a wide tail pool (6 banks)
        with tc.tile_pool(name="ps_tail", bufs=6, space="PSUM") as ps_tail:
            ps_ref[0] = ps_tail
            # ---- tail: ic1 MLP ----
            mu1 = emit_ln2_mu(1, 0, 512)
            for mo in range(DC):
                emit_proj_chunk(1, mo, 0, 512)
            bcs = emit_ln2_s2(1, 0, 256, *mu1)
            bcs_b1 = emit_ln2_s2(1, 0, 512, *mu1)
            emit_ln2_apply(1, bcs, 0, 256, range(DC))
            # band0 gelus go to a buffer aliased on dead v_aug memory, so
            # fc1(ic1) needn't wait for fc2(ic0) to drain gelu_t
            gelu_b0 = mem.tile([P, HC, 256], FP8, tag="vf", name="gelu_b0")
            for mo in range(HC):
                emit_fc1_chunk(1, mo, 0, 256, gdst=gelu_b0[:, mo, :])
                if mo % 2 == 1 and mo < 12:
                    emit_fc2_chunk(0, mo // 2, 0, 512)
                elif mo >= 12 and mo % 3 == 2:
                    emit_exit_tr((mo - 12) // 3)
                if mo == 11:
                    emit_ln2_apply(1, bcs_b1, 256, 256, range(DC))
            for mo in range(HC):
                emit_fc1_chunk(1, mo, 256, 256,
                               gdst=gelu_t[:, mo, 0:256])
                if mo % 4 == 3:
                    emit_fc2_chunk(
                        1, mo // 4, 0, 256,
                        gsrc=lambda kp: gelu_b0[:, 2 * kp:2 * kp + 2, :])
            for t in (4, 5):
                emit_exit_tr(t, tail=True)
            for mo in range(DC):
                emit_fc2_chunk(
                    1, mo, 256, 256,
                    gsrc=lambda kp: gelu_t[:, 2 * kp:2 * kp + 2, 0:256])
            for t in (6, 7):
                emit_exit_tr(t, tail=True)



_NC_CACHE = {}


def _get_nc(qkb_zero=False):
    key = ("nc", qkb_zero)
    if key not in _NC_CACHE:
        _NC_CACHE[key] = build_nc(qkb_zero)
    return _NC_CACHE[key]


def _fold_perm():
    perm = []
    for g in range(3):
        for half in range(2):
            for hh in range(4):
                h = 4 * g + hh
                perm.extend(range(h * 64 + 32 * half, h * 64 + 32 * half + 32))
    return np.asarray(perm)


def _prep_inputs(x, ln1_g, ln1_b, qkv_w, qkv_b, proj_w, proj_b,
                 ln2_g, ln2_b, fc1_w, fc1_b, fc2_w, fc2_b):
    f = lambda a: np.asarray(a, np.float32)
    x = f(x)
    qkv_w, qkv_b = f(qkv_w), f(qkv_b)
    proj_w, proj_b = f(proj_w), f(proj_b)
    fc1_w, fc1_b = f(fc1_w), f(fc1_b)
    fc2_w, fc2_b = f(fc2_w), f(fc2_b)
    ln1_g, ln1_b, ln2_g, ln2_b = f(ln1_g), f(ln1_b), f(ln2_g), f(ln2_b)

    # fold LN affine into the following matmul
    qkv_w_eff = ln1_g[:, None] * qkv_w
    qkv_b_eff = qkv_b + ln1_b @ qkv_w
    fc1_w_eff = ln2_g[:, None] * fc1_w
    fc1_b_eff = (fc1_b + ln2_b @ fc1_w).astype(np.float32)

    # v bias commutes through softmax -> fold into proj bias
    vb = qkv_b_eff[2 * D:]
    proj_b_eff = (proj_b + vb @ proj_w).astype(np.float32)

    # fold permutation for q/k DoubleRow scores
    perm = _fold_perm()
    wq = qkv_w_eff[:, 0:D][:, perm]
    wk = qkv_w_eff[:, D:2 * D][:, perm]
    wv = qkv_w_eff[:, 2 * D:]
    wqkv = np.concatenate([wq, wk, wv], axis=1).astype(FP8_NP)
    qkb = np.concatenate([qkv_b_eff[0:D][perm],
                          qkv_b_eff[D:2 * D][perm]]).astype(np.float32)

    proj_w8 = proj_w.astype(FP8_NP)
    wsum8 = np.repeat(proj_w8.astype(np.float32).sum(axis=1, keepdims=True),
                      32, axis=1).astype(FP8_NP)
    pbsum = proj_b_eff.sum() / D
    biases = np.concatenate([
        qkb, proj_b_eff, fc1_b_eff, fc2_b.astype(np.float32),
        np.full(P, pbsum, np.float32)]).astype(np.float32)
    shared = {
        "wqkv": wqkv, "wsum8": wsum8, "biases": biases,
        "proj_w": proj_w8,
        "fc1_w": fc1_w_eff[:, HID // 2:].astype(BF16_NP),
        "fc1_w8": (16.0 * fc1_w_eff[:, :HID // 2]).astype(FP8_NP),
        "fc2_w": (16.0 * fc2_w).astype(FP8_NP),
    }
    n_cores = x.shape[0]
    return [{"x": np.ascontiguousarray(x[c]).astype(BF16_NP), **shared}
            for c in range(n_cores)]


def kernel(**inputs):
    in_maps = _prep_inputs(**inputs)
    nc = _get_nc(
        qkb_zero=bool(np.all(in_maps[0]["biases"][:2 * D] == 0.0)))
    res = run_bass_kernel_spmd(nc, in_maps, core_ids=list(range(len(in_maps))))
    return np.stack([r["y"] for r in res.results], axis=0)


if __name__ == "__main__":
    import reference
    inputs = {k: np.asarray(v) for k, v in reference.setup_inputs().items()}
    out = kernel(**inputs)
    print("kernel out", out.shape, out.dtype)

